# revision 13
# baseline (speedup 1.0000x reference)
"""Trainium2 Bass kernel for relu-kernelized multi-head attention with a
per-head Toeplitz relative-position mask (sparse_attention problem).

Contract: kernel(**inputs) takes FULL unsharded inputs (numpy), returns the
FULL output [16, 1025, 768]. Internally: data-parallel over batch across 8
NeuronCores (2 batches/core), identical SPMD program, per-core inputs differ
only in the x shard.

Math (per batch b):
  q = relu((x@wq + bq)/8) + eps ; k = relu(x@wk + bk) + eps ; v = x@wv + bv
  S[q,k] = sum_d q*k ;  attn = S*|tm| + eps ; attn /= rowsum ; out = attn@v
  y = out@wo + bo

v2: all matmul operands in bf16 (PE runs 1 cycle/row vs fp32's 4; mask DMA
halves).  PSUM accumulation stays fp32.  The q/k "+eps" is dropped (its
effect is ~1e-7 relative; the attention-level eps is kept exactly via the
cs_cols rank-1 correction and the rowsum + L*eps denominator).  The row
normalization is batched per head-pair: 4 rowsum rows are gathered into one
[4, L] tile, one reciprocal_approx_fast (~51 ULP, fine vs the 2e-2 gate)
replaces 4 serial [1, L] full-precision reciprocals.  Attention outputs stay
resident in SBUF as 12 [128, L] bf16 head-pair tiles consumed directly by
the output projection (no DRAM spill), with bo fused into the drain
activation.

Device-side layout choices:
  - x shipped transposed+padded with a ones-row: xaT [2, 769, 1152] so the
    V bias folds into the matmul as a K=1 extra contraction chunk.
  - qT/kT produced in [head*64, token] layout -> S^T tiles [k,q] come
    straight from matmuls with K=d=64.
  - mask |tm| is gathered on host (pure input preprocessing: a
    Toeplitz-strided view of toeplitz_params), shipped transposed
    [h, k, q] in bf16, padded with zeros on the k dim.
  - v_aug [token, 65] per head carries a ones column: the AV matmul's row 64
    accumulates the rowsum for free.  The "+eps" of the reference rides in
    as a rank-1 correction: eps * colsum(v_aug), added during normalization.
"""

import os
import sys

sys.path.insert(0, "/opt/trn_rl_repo")

import numpy as np

B, L, F, H, D = 16, 1025, 768, 12, 64
NB = 32
EPS = 1e-8
LP = 1152           # padded token count (9 * 128)
NKB = 9             # k blocks of 128
QM = 1024           # main q width (q tail = 1 col, index 1024)
FA = F + 1          # augmented contraction (ones row)
NCORES = 8
BPC = B // NCORES   # batches per core

_PROG = None


def _build_program():
    import concourse.bass as bass
    import concourse.tile as tile
    from concourse import mybir

    f32 = mybir.dt.float32
    bf16 = mybir.dt.bfloat16
    AF = mybir.ActivationFunctionType

    nc = bass.Bass()

    xaT = nc.declare_dram_parameter("xaT", [BPC, FA, LP], bf16, isOutput=False)
    wq_aug = nc.declare_dram_parameter("wq_aug", [FA, F], bf16, isOutput=False)
    wk_aug = nc.declare_dram_parameter("wk_aug", [FA, F], bf16, isOutput=False)
    wv_aug = nc.declare_dram_parameter("wv_aug", [FA, H * 65], bf16, isOutput=False)
    wo_flat = nc.declare_dram_parameter("wo_flat", [H * D, F], bf16, isOutput=False)
    bo_in = nc.declare_dram_parameter("bo", [F], f32, isOutput=False)
    mask_main = nc.declare_dram_parameter(
        "maskT_main", [H, NKB, 128, QM], bf16, isOutput=False
    )
    mask_tail = nc.declare_dram_parameter(
        "maskT_tail", [H, 128, NKB], bf16, isOutput=False
    )
    yT = nc.declare_dram_parameter("yT", [BPC, F, L], f32, isOutput=True)

    rr_dram = nc.dram_tensor("rr_dram", [8, L], f32)
    bqk = nc.declare_dram_parameter("bqk_eff", [2, F], f32, isOutput=False)
    cs_in = nc.declare_dram_parameter("cs_cols", [BPC, 2, 65, 6], f32, isOutput=False)

    with tile.TileContext(nc) as tc:
        from contextlib import ExitStack

        with ExitStack() as octx:
            consts = octx.enter_context(tc.tile_pool(name="consts", bufs=1))
            # attention outputs, SBUF-resident across phases: 12 tiles
            # [128, L] bf16, one per (batch, head-pair); rows 0:64 = even
            # head, 64:128 = odd head of the pair
            ot_pool = octx.enter_context(tc.tile_pool(name="ot", bufs=2 * 6))
            wo_pool = octx.enter_context(tc.tile_pool(name="wo", bufs=6))
            bo_pool = octx.enter_context(tc.tile_pool(name="bo", bufs=1))
            ctx = octx.enter_context(ExitStack())
            xa_pool = ctx.enter_context(tc.tile_pool(name="xa", bufs=2 * 6))
            wqk_pool = ctx.enter_context(tc.tile_pool(name="wqk", bufs=2))
            wv_pool = ctx.enter_context(tc.tile_pool(name="wv", bufs=2))
            qkt_pool = ctx.enter_context(tc.tile_pool(name="qkt", bufs=2))
            vaug_pool = ctx.enter_context(tc.tile_pool(name="vaug", bufs=4))
            csc_pool = ctx.enter_context(tc.tile_pool(name="cscol", bufs=2))
            bias_pool = ctx.enter_context(tc.tile_pool(name="bias", bufs=2))
            mask_pool = ctx.enter_context(tc.tile_pool(name="mask", bufs=12))
            mtail_pool = ctx.enter_context(tc.tile_pool(name="mtail", bufs=2))
            mt_pool = ctx.enter_context(tc.tile_pool(name="mt", bufs=4))
            mttail_pool = ctx.enter_context(tc.tile_pool(name="mttail", bufs=2))
            rs_pool = ctx.enter_context(tc.tile_pool(name="rs", bufs=1))
            rrb_pool = ctx.enter_context(tc.tile_pool(name="rrb", bufs=4))
            avsb_pool = ctx.enter_context(tc.tile_pool(name="avsb", bufs=5))

            ps_proj = ctx.enter_context(
                tc.tile_pool(name="ps_proj", bufs=2, space="PSUM")
            )
            ps_s = ctx.enter_context(tc.tile_pool(name="ps_s", bufs=3, space="PSUM"))
            ps_av = ctx.enter_context(tc.tile_pool(name="ps_av", bufs=1, space="PSUM"))
            ps_tails = ctx.enter_context(
                tc.tile_pool(name="ps_tails", bufs=1, space="PSUM")
            )

            dma = nc.sync
            dma2 = nc.gpsimd  # second DMA-issue queue for the normalize path

            # constants
            ones_row = consts.tile([1, LP], bf16)
            nc.vector.memset(ones_row[:, 0:L], 1.0)
            nc.vector.memset(ones_row[:, L:LP], 0.0)

            ot_pairs = {}
            for b in range(BPC):
                for pair in range(6):
                    ot_pairs[(b, pair)] = ot_pool.tile(
                        [128, L], bf16, tag="ot", name="ot_pair"
                    )

            # ---- persistent xaT in SBUF --------------------------------
            # 6 full 128-row chunks per batch + the ones-row (row 768)
            xa = {}
            for b in range(BPC):
                for c in range(6):
                    t = xa_pool.tile([128, LP], bf16, tag="xa", name="xa_tile")
                    dma.dma_start(out=t, in_=xaT[b, c * 128 : (c + 1) * 128, :])
                    xa[(b, c)] = t
            for b in range(BPC):
                xa[(b, 6)] = ones_row

            # output-projection weights, prefetched so the O phase starts
            # without a DMA stall
            bo_sb = bo_pool.tile([128, 6], f32)
            for fc in range(6):
                dma.dma_start(
                    out=bo_sb[:, fc : fc + 1], in_=bo_in[fc * 128 : (fc + 1) * 128]
                )
            wo_sb = []
            for hc in range(6):
                t = wo_pool.tile([128, F], bf16, tag="wo", name="wo_tile")
                dma.dma_start(out=t, in_=wo_flat[hc * 128 : (hc + 1) * 128, :])
                wo_sb.append(t)

            # q sub-tiles for projections (moving dim <= 512)
            qsubs = [(0, 512), (512, 512), (1024, 128)]
            # attention q tiling: main [0,1024) in 2 psum-bank halves + tail col
            def st_slices():
                return [(0, 512), (512, 512)]

            # ---- v projections + colsums, per 3-pair group --------------
            # wv_aug columns are grouped per head: h*65 + (0..63 -> wv, 64 -> ones)
            vaug = {}      # (b, g) -> [128, NKB, 390]
            csum = {}      # (b, g) -> [65, 6]

            def emit_vproj(g):
                wv_sb = wv_pool.tile([128, 7, 390], bf16, tag="wv")
                c0 = g * 390
                for c in range(6):
                    dma.dma_start(
                        out=wv_sb[:, c, :],
                        in_=wv_aug[c * 128 : (c + 1) * 128, c0 : c0 + 390],
                    )
                dma.dma_start(
                    out=wv_sb[0:1, 6, :], in_=wv_aug[F : F + 1, c0 : c0 + 390]
                )
                for b in range(BPC):
                    va = vaug_pool.tile([128, NKB, 390], bf16, tag="vaug")
                    for tb in range(NKB):
                        ps = ps_proj.tile([128, 512], f32, tag="ps_p", name="ps_v")
                        for c in range(6):
                            nc.tensor.matmul(
                                ps[:, 0:390],
                                xa[(b, c)][:, tb * 128 : (tb + 1) * 128],
                                wv_sb[:, c, :],
                                start=(c == 0),
                                stop=False,
                            )
                        nc.tensor.matmul(
                            ps[:, 0:390],
                            xa[(b, 6)][:, tb * 128 : (tb + 1) * 128],
                            wv_sb[0:1, 6, :],
                            start=False,
                            stop=True,
                        )
                        nc.scalar.activation(va[:, tb, :], ps[:, 0:390], AF.Copy)
                    vaug[(b, g)] = va
                    cs_col = csc_pool.tile([65, 6], f32, tag="cscol")
                    dma2.dma_start(out=cs_col, in_=cs_in[b, g])
                    csum[(b, g)] = cs_col

            # ---- main loop over head pairs ------------------------------
            for pair in range(6):
                g = pair // 3
                if pair % 3 == 0:
                    emit_vproj(g)

                # qT/kT projections for this pair, both batches
                wq_sb = wqk_pool.tile([128, 6, 128], bf16, tag="wq")
                wk_sb = wqk_pool.tile([128, 6, 128], bf16, tag="wk")
                p0 = pair * 128
                for c in range(6):
                    dma.dma_start(
                        out=wq_sb[:, c, :],
                        in_=wq_aug[c * 128 : (c + 1) * 128, p0 : p0 + 128],
                    )
                    dma.dma_start(
                        out=wk_sb[:, c, :],
                        in_=wk_aug[c * 128 : (c + 1) * 128, p0 : p0 + 128],
                    )
                bq_sb = bias_pool.tile([128, 2], f32, tag="bqk")
                dma.dma_start(out=bq_sb[:, 0:1], in_=bqk[0, p0 : p0 + 128])
                dma.dma_start(out=bq_sb[:, 1:2], in_=bqk[1, p0 : p0 + 128])

                qT = {}
                kT = {}
                for b in range(BPC):
                    qt = qkt_pool.tile([128, LP], bf16, tag="qT")
                    kt = qkt_pool.tile([128, LP], bf16, tag="kT")
                    for (dst, w_sb, scl, bi) in (
                        (qt, wq_sb, 0.125, 0),
                        (kt, wk_sb, 1.0, 1),
                    ):
                        for (q0, qw) in qsubs:
                            psq = ps_proj.tile(
                                [128, 512], f32, tag="ps_p", name="ps_qk"
                            )
                            for c in range(6):
                                nc.tensor.matmul(
                                    psq[:, 0:qw],
                                    w_sb[:, c, :],
                                    xa[(b, c)][:, q0 : q0 + qw],
                                    start=(c == 0), stop=(c == 5),
                                )
                            # relu(scale*xw + scale*b); the reference's +eps
                            # here is dropped (~1e-7 relative effect)
                            nc.scalar.activation(
                                dst[:, q0 : q0 + qw], psq[:, 0:qw], AF.Relu,
                                scale=scl, bias=bq_sb[:, bi : bi + 1],
                            )
                    qT[b] = qt
                    kT[b] = kt

                av_sbs = {}
                for hh in range(2):
                    h = pair * 2 + hh
                    r0 = hh * 64
                    # mask tiles for this head (shared across batches)
                    mks = []
                    for j in range(NKB):
                        mk = mask_pool.tile(
                            [128, QM], bf16, tag="mask", name="mask_tile"
                        )
                        dma.dma_start(out=mk, in_=mask_main[h, j])
                        mks.append(mk)
                    mkt = mtail_pool.tile([128, NKB], bf16, tag="mtail")
                    dma.dma_start(out=mkt, in_=mask_tail[h])

                    for b in range(BPC):
                        va = vaug[(b, pair // 3)]
                        vc0 = (pair % 3) * 130 + hh * 65

                        av = ps_av.tile([65, QM], f32, tag="ps_av")
                        ptl = ps_tails.tile([128, 16], f32, tag="ps_tails")
                        stail = ptl[:, 0:NKB]
                        avt = ptl[0:65, NKB : NKB + 1]
                        mtt = mttail_pool.tile([128, NKB], bf16, tag="mttail")

                        for j in range(NKB):
                            lhs_k = kT[b][r0 : r0 + 64, j * 128 : (j + 1) * 128]
                            mthalf = []
                            for (q0, qw) in st_slices():
                                st = ps_s.tile([128, 512], f32, tag="ps_s")
                                nc.tensor.matmul(
                                    st,
                                    lhs_k,
                                    qT[b][r0 : r0 + 64, q0 : q0 + qw],
                                    start=True, stop=True,
                                )
                                # masked scores -> bf16
                                mt = mt_pool.tile([128, 512], bf16, tag="mt")
                                nc.vector.tensor_mul(
                                    mt, st, mks[j][:, q0 : q0 + qw]
                                )
                                mthalf.append((q0, qw, mt))
                            # tail column q=1024 (shares the kT weights)
                            nc.tensor.matmul(
                                stail[:, j : j + 1],
                                lhs_k,
                                qT[b][r0 : r0 + 64, QM : QM + 1],
                                start=True, stop=True,
                            )
                            # AV accumulation (row 64 = rowsum via ones col)
                            for (q0, qw, mt) in mthalf:
                                nc.tensor.matmul(
                                    av[:, q0 : q0 + qw],
                                    va[:, j, vc0 : vc0 + 65],
                                    mt,
                                    start=(j == 0), stop=(j == NKB - 1),
                                )
                        # tail: masked scores + AV
                        nc.vector.tensor_mul(mtt, stail, mkt)
                        for j in range(NKB):
                            nc.tensor.matmul(
                                avt,
                                va[:, j, vc0 : vc0 + 65],
                                mtt[:, j : j + 1],
                                start=(j == 0), stop=(j == NKB - 1),
                            )

                        # drain AV psum to SBUF (frees the banks for the
                        # next head while the normalize chain runs)
                        av_sb = avsb_pool.tile([65, L], f32, tag="avsb")
                        nc.scalar.activation(av_sb[:, 0:QM], av, AF.Copy)
                        nc.scalar.activation(av_sb[:, QM : QM + 1], avt, AF.Copy)
                        av_sbs[(hh, b)] = av_sb

                # ---- batched normalization for the pair's 4 (hh, b) -----
                # gather the 4 rowsum rows -> [4, L], one approx reciprocal
                rs4 = rs_pool.tile([4, L], f32, tag="rs")
                order = [(hh, b) for hh in range(2) for b in range(BPC)]
                for idx, (hh, b) in enumerate(order):
                    dma2.dma_start(
                        out=rs4[idx : idx + 1, :],
                        in_=av_sbs[(hh, b)][64:65, :],
                    )
                nc.vector.tensor_scalar_add(rs4, rs4, float(L) * EPS)
                rr4 = rs_pool.tile([4, L], f32, tag="rr")
                nc.vector.reciprocal(rr4, rs4)
                s0 = (pair % 2) * 4
                dma2.dma_start(out=rr_dram[s0 : s0 + 4, :], in_=rr4)
                for idx, (hh, b) in enumerate(order):
                    rr_slot = rr_dram[s0 + idx]
                    rr_bcast_src = bass.AP(
                        tensor=rr_slot.tensor,
                        offset=rr_slot.offset,
                        ap=[[0, 64]] + list(rr_slot.ap),
                    )
                    rrb = rrb_pool.tile([64, L], f32, tag="rrb")
                    dma2.dma_start(out=rrb, in_=rr_bcast_src)
                    hg = (pair % 3) * 2 + hh
                    cs = csum[(b, pair // 3)]
                    r0 = hh * 64
                    nc.vector.scalar_tensor_tensor(
                        ot_pairs[(b, pair)][r0 : r0 + 64, :],
                        av_sbs[(hh, b)][0:64, :],
                        cs[0:64, hg : hg + 1],
                        rrb,
                        op0=mybir.AluOpType.add,
                        op1=mybir.AluOpType.mult,
                    )

            # ---- output projection: yT = wo^T @ O^T + bo ----------------
            ctx.close()
            ctx = octx.enter_context(ExitStack())
            y_pool = ctx.enter_context(tc.tile_pool(name="y", bufs=7))
            ps_y = ctx.enter_context(tc.tile_pool(name="ps_y", bufs=2, space="PSUM"))

            oq_tiles = [(0, 512), (512, 512), (1024, 1)]
            for b in range(BPC):
                ys = []
                for fc in range(6):
                    ys.append(y_pool.tile([128, L], f32, tag="y", name="y_tile"))
                for (q0, qw) in oq_tiles:
                    for fc in range(6):
                        psy = ps_y.tile([128, 512], f32, tag="ps_y")
                        for hc in range(6):
                            nc.tensor.matmul(
                                psy[:, 0:qw],
                                wo_sb[hc][:, fc * 128 : (fc + 1) * 128],
                                ot_pairs[(b, hc)][:, q0 : q0 + qw],
                                start=(hc == 0), stop=(hc == 5),
                            )
                        # drain with bo fused as the per-partition bias
                        nc.scalar.activation(
                            ys[fc][:, q0 : q0 + qw], psy[:, 0:qw], AF.Identity,
                            bias=bo_sb[:, fc : fc + 1],
                        )
                for fc in range(6):
                    dma.dma_start(
                        out=yT[b, fc * 128 : (fc + 1) * 128, :], in_=ys[fc]
                    )

    _split_matmul_waits(nc)
    return nc


def _split_matmul_waits(nc):
    """Walrus TPB instruction structs encode a limited number of sync waits
    (the fp32 LDWEIGHTS+MATMUL pair can take none beyond its update).  Hoist
    excess waits onto same-engine NoOps inserted just before each
    instruction."""
    import bass_rust
    from concourse import mybir

    n = 0
    for f in nc.m.functions:
        for blk in f.blocks:
            insts = blk.instructions
            out = []
            for inst in insts:
                si = inst.sync_info
                tname = type(inst).__name__
                if si is not None and len(si.on_wait) > 0 and "ISA" not in tname:
                    cap = 0 if tname == "InstMatmult" else 1
                    waits = list(si.on_wait)
                    if len(waits) > cap:
                        hoist = waits[: len(waits) - cap]
                        keep = waits[len(waits) - cap :]
                        for w in hoist:
                            nop = mybir.InstNoOp(
                                name=f"I-mmw-{n}", ins=[], outs=[]
                            )
                            n += 1
                            nop.engine = inst.engine
                            nop.sync_info = bass_rust.SyncInfo(
                                on_wait=[w], on_update=[]
                            )
                            out.append(nop)
                        inst.sync_info = bass_rust.SyncInfo(
                            on_wait=keep, on_update=list(si.on_update)
                        )
                out.append(inst)
            insts[:] = out
    return n


def _dist_index():
    gi = np.arange(NB)
    gj = np.arange(NB)
    idx = (
        (gi[:, None, None, None] - gi[None, None, :, None] + NB) * 2 * NB
        + gj[None, :, None, None]
        - gj[None, None, None, :]
        + NB
    )
    return idx.reshape(-1).astype(np.int32)


def _host_prep(x, wq, bq, wk, bk, wv, bv, wo, bo, toeplitz_params):
    import ml_dtypes

    f4 = np.float32
    bf = ml_dtypes.bfloat16
    x = np.asarray(x, f4)
    L0 = NB * NB

    xaT = np.zeros((B, FA, LP), bf)
    xaT[:, :F, :L] = np.transpose(x, (0, 2, 1)).astype(bf)
    xaT[:, F, :L] = 1.0

    wq_aug = np.empty((FA, F), bf)
    wq_aug[:F] = np.asarray(wq, f4).reshape(F, F).astype(bf)
    wq_aug[F] = np.asarray(bq, f4).reshape(F).astype(bf)
    wk_aug = np.empty((FA, F), bf)
    wk_aug[:F] = np.asarray(wk, f4).reshape(F, F).astype(bf)
    wk_aug[F] = np.asarray(bk, f4).reshape(F).astype(bf)

    wv_aug = np.zeros((FA, H * 65), bf)
    wvr = np.asarray(wv, f4)
    bvr = np.asarray(bv, f4)
    for h in range(H):
        wv_aug[:F, h * 65 : h * 65 + 64] = wvr[:, h, :].astype(bf)
        wv_aug[F, h * 65 : h * 65 + 64] = bvr[h].astype(bf)
        wv_aug[F, h * 65 + 64] = 1.0

    wo_flat = np.ascontiguousarray(
        np.asarray(wo, f4).reshape(H * D, F).astype(bf)
    )
    bo_arr = np.asarray(bo, f4).reshape(F)

    # gathered |toeplitz| mask, padded (CLS row/col of ones), transposed,
    # k padded to 1152 with zeros
    tp = np.asarray(toeplitz_params, f4)
    tm = np.abs(tp[:, _dist_index()]).reshape(H, L0, L0)
    tm_full = np.ones((H, L, L), f4)
    tm_full[:, 1:, 1:] = tm
    maskT = np.zeros((H, LP, L), bf)
    maskT[:, :L, :] = np.transpose(tm_full, (0, 2, 1)).astype(bf)
    maskT_main = np.ascontiguousarray(
        maskT[:, :, :QM].reshape(H, NKB, 128, QM)
    )
    maskT_tail = np.ascontiguousarray(
        maskT[:, :, QM].reshape(H, NKB, 128).transpose(0, 2, 1)
    )

    xsum = x[:, :, :].sum(axis=1)  # [B, F]
    cs = np.einsum("bf,fhd->bhd", xsum, wvr) + L * bvr[None]  # [B, H, 64]
    cs_full = np.concatenate(
        [cs, np.full((B, H, 1), float(L), np.float32)], axis=2
    ) * np.float32(EPS)  # [B, H, 65]
    cs_cols = np.zeros((B, 2, 65, 6), f4)
    for g in range(2):
        for hh in range(6):
            cs_cols[:, g, :, hh] = cs_full[:, 6 * g + hh, :]
    bqk_eff = np.stack(
        [np.asarray(bq, f4).reshape(F) * 0.125, np.asarray(bk, f4).reshape(F)]
    )
    shared = dict(
        bqk_eff=bqk_eff,
        wq_aug=wq_aug,
        wk_aug=wk_aug,
        wv_aug=wv_aug,
        wo_flat=wo_flat,
        bo=bo_arr,
        maskT_main=maskT_main,
        maskT_tail=maskT_tail,
    )
    in_maps = []
    for c in range(NCORES):
        m = dict(shared)
        m["xaT"] = np.ascontiguousarray(xaT[c * BPC : (c + 1) * BPC])
        m["cs_cols"] = np.ascontiguousarray(cs_cols[c * BPC : (c + 1) * BPC])
        in_maps.append(m)
    return in_maps


def _get_program():
    global _PROG
    if _PROG is None:
        _PROG = _build_program()
    return _PROG


def run(trace=False, **inputs):
    from concourse.bass_utils import run_bass_kernel_spmd

    nc = _get_program()
    in_maps = _host_prep(**inputs)
    res = run_bass_kernel_spmd(nc, in_maps, list(range(NCORES)), trace=trace)
    outs = []
    for c in range(NCORES):
        yt = res.results[c]["yT"]  # [BPC, F, L]
        outs.append(np.transpose(yt, (0, 2, 1)))
    y = np.concatenate(outs, axis=0).astype(np.float32)
    return y, res


def kernel(**inputs):
    y, _ = run(trace=False, **inputs)
    return y


# revision 22
# speedup vs baseline: 1.1212x; 1.1212x over previous
"""Trainium2 Bass kernel for relu-kernelized multi-head attention with a
per-head Toeplitz relative-position mask (sparse_attention problem).

Contract: kernel(**inputs) takes FULL unsharded inputs (numpy), returns the
FULL output [16, 1025, 768]. Internally: data-parallel over batch across 8
NeuronCores (2 batches/core), identical SPMD program, per-core inputs differ
only in the x shard.

Math (per batch b):
  q = relu((x@wq + bq)/8) + eps ; k = relu(x@wk + bk) + eps ; v = x@wv + bv
  S[q,k] = sum_d q*k ;  attn = S*|tm| + eps ; attn /= rowsum ; out = attn@v
  y = out@wo + bo

v2: all matmul operands in bf16 (PE runs 1 cycle/row vs fp32's 4; mask DMA
halves).  PSUM accumulation stays fp32.  The q/k "+eps" is dropped (its
effect is ~1e-7 relative; the attention-level eps is kept exactly via the
cs_cols rank-1 correction and the rowsum + L*eps denominator).  The row
normalization is batched per head-pair: 4 rowsum rows are gathered into one
[4, L] tile, one reciprocal_approx_fast (~51 ULP, fine vs the 2e-2 gate)
replaces 4 serial [1, L] full-precision reciprocals.  Attention outputs stay
resident in SBUF as 12 [128, L] bf16 head-pair tiles consumed directly by
the output projection (no DRAM spill), with bo fused into the drain
activation.

Device-side layout choices:
  - x shipped transposed+padded with a ones-row: xaT [2, 769, 1152] so the
    V bias folds into the matmul as a K=1 extra contraction chunk.
  - qT/kT produced in [head*64, token] layout -> S^T tiles [k,q] come
    straight from matmuls with K=d=64.
  - mask |tm| is gathered on host (pure input preprocessing: a
    Toeplitz-strided view of toeplitz_params), shipped transposed
    [h, k, q] in bf16, padded with zeros on the k dim.
  - v_aug [token, 65] per head carries a ones column: the AV matmul's row 64
    accumulates the rowsum for free.  The "+eps" of the reference rides in
    as a rank-1 correction: eps * colsum(v_aug), added during normalization.
"""

import os
import sys

sys.path.insert(0, "/opt/trn_rl_repo")

import numpy as np

B, L, F, H, D = 16, 1025, 768, 12, 64
NB = 32
EPS = 1e-8
LP = 1152           # padded token count (9 * 128)
NKB = 9             # k blocks of 128
QM = 1024           # main q width (q tail = 1 col, index 1024)
FA = F + 1          # augmented contraction (ones row)
NCORES = 8
BPC = B // NCORES   # batches per core

_PROG = None


def _build_program():
    import concourse.bass as bass
    import concourse.tile as tile
    from concourse import mybir

    f32 = mybir.dt.float32
    bf16 = mybir.dt.bfloat16
    AF = mybir.ActivationFunctionType

    nc = bass.Bass()

    xaT = nc.declare_dram_parameter("xaT", [BPC, FA, LP], bf16, isOutput=False)
    wq_aug = nc.declare_dram_parameter("wq_aug", [FA, F], bf16, isOutput=False)
    wk_aug = nc.declare_dram_parameter("wk_aug", [FA, F], bf16, isOutput=False)
    wv_aug = nc.declare_dram_parameter("wv_aug", [FA, H * 65], bf16, isOutput=False)
    wo_flat = nc.declare_dram_parameter("wo_flat", [H * D, F], bf16, isOutput=False)
    bo_in = nc.declare_dram_parameter("bo", [F], f32, isOutput=False)
    mask_main = nc.declare_dram_parameter(
        "maskT_main", [H, NKB, 128, QM], bf16, isOutput=False
    )
    mask_tail = nc.declare_dram_parameter(
        "maskT_tail", [H, 128, NKB], bf16, isOutput=False
    )
    yT = nc.declare_dram_parameter("yT", [BPC, F, L], f32, isOutput=True)

    rr_dram = nc.dram_tensor("rr_dram", [8, L], f32)
    bqk = nc.declare_dram_parameter("bqk_eff", [2, F], f32, isOutput=False)
    cs_in = nc.declare_dram_parameter("cs_cols", [BPC, 2, 65, 6], f32, isOutput=False)

    with tile.TileContext(nc) as tc:
        from contextlib import ExitStack

        with ExitStack() as octx:
            consts = octx.enter_context(tc.tile_pool(name="consts", bufs=1))
            # attention outputs, SBUF-resident across phases: 12 tiles
            # [128, L] bf16, one per (batch, head-pair); rows 0:64 = even
            # head, 64:128 = odd head of the pair
            ot_pool = octx.enter_context(tc.tile_pool(name="ot", bufs=2 * 6))
            wo_pool = octx.enter_context(tc.tile_pool(name="wo", bufs=6))
            bo_pool = octx.enter_context(tc.tile_pool(name="bo", bufs=1))
            ctx = octx.enter_context(ExitStack())
            xa_pool = ctx.enter_context(tc.tile_pool(name="xa", bufs=2 * 6))
            wqk_pool = ctx.enter_context(tc.tile_pool(name="wqk", bufs=2))
            wv_pool = ctx.enter_context(tc.tile_pool(name="wv", bufs=2))
            qkt_pool = ctx.enter_context(tc.tile_pool(name="qkt", bufs=2))
            vaug_pool = ctx.enter_context(tc.tile_pool(name="vaug", bufs=4))
            csc_pool = ctx.enter_context(tc.tile_pool(name="cscol", bufs=2))
            bias_pool = ctx.enter_context(tc.tile_pool(name="bias", bufs=2))
            mask_pool = ctx.enter_context(tc.tile_pool(name="mask", bufs=12))
            mtail_pool = ctx.enter_context(tc.tile_pool(name="mtail", bufs=2))
            mt_pool = ctx.enter_context(tc.tile_pool(name="mt", bufs=3))
            mttail_pool = ctx.enter_context(tc.tile_pool(name="mttail", bufs=2))
            rs_pool = ctx.enter_context(tc.tile_pool(name="rs", bufs=1))
            rrb_pool = ctx.enter_context(tc.tile_pool(name="rrb", bufs=4))
            avsb_pool = ctx.enter_context(tc.tile_pool(name="avsb", bufs=5))

            # flex pool: [128,512] tiles time-shared between projection psums
            # (2-deep so the activation drain doesn't stall the next matmul
            # group) and the per-head tail psum (stail+avt live in a slice)
            ps_flex = ctx.enter_context(
                tc.tile_pool(name="ps_flex", bufs=2, space="PSUM")
            )
            ps_s = ctx.enter_context(tc.tile_pool(name="ps_s", bufs=2, space="PSUM"))
            ps_av = ctx.enter_context(tc.tile_pool(name="ps_av", bufs=1, space="PSUM"))

            dma = nc.sync
            dma2 = nc.gpsimd  # second DMA-issue queue for the normalize path

            # constants
            ones_row = consts.tile([1, LP], bf16)
            nc.vector.memset(ones_row[:, 0:L], 1.0)
            nc.vector.memset(ones_row[:, L:LP], 0.0)

            ot_pairs = {}
            for b in range(BPC):
                for pair in range(6):
                    ot_pairs[(b, pair)] = ot_pool.tile(
                        [128, L], bf16, tag="ot", name="ot_pair"
                    )

            # ---- persistent xaT in SBUF --------------------------------
            # 6 full 128-row chunks per batch + the ones-row (row 768)
            xa = {}
            for b in range(BPC):
                for c in range(6):
                    t = xa_pool.tile([128, LP], bf16, tag="xa", name="xa_tile")
                    dma.dma_start(out=t, in_=xaT[b, c * 128 : (c + 1) * 128, :])
                    xa[(b, c)] = t
            for b in range(BPC):
                xa[(b, 6)] = ones_row

            # output-projection weights, prefetched so the O phase starts
            # without a DMA stall
            bo_sb = bo_pool.tile([128, 6], f32)
            for fc in range(6):
                dma.dma_start(
                    out=bo_sb[:, fc : fc + 1], in_=bo_in[fc * 128 : (fc + 1) * 128]
                )
            wo_sb = []
            for hc in range(6):
                t = wo_pool.tile([128, F], bf16, tag="wo", name="wo_tile")
                dma.dma_start(out=t, in_=wo_flat[hc * 128 : (hc + 1) * 128, :])
                wo_sb.append(t)

            # q sub-tiles for projections (moving dim <= 512)
            qsubs = [(0, 512), (512, 512), (1024, 128)]
            # attention q tiling: main [0,1024) in 2 psum-bank halves + tail col
            def st_slices():
                return [(0, 512), (512, 512)]

            # ---- v projections + colsums, per 3-pair group --------------
            # wv_aug columns are grouped per head: h*65 + (0..63 -> wv, 64 -> ones)
            vaug = {}      # (b, g) -> [128, NKB, 390]
            csum = {}      # (b, g) -> [65, 6]

            def emit_vproj(g):
                wv_sb = wv_pool.tile([128, 7, 390], bf16, tag="wv")
                c0 = g * 390
                for c in range(6):
                    dma.dma_start(
                        out=wv_sb[:, c, :],
                        in_=wv_aug[c * 128 : (c + 1) * 128, c0 : c0 + 390],
                    )
                dma.dma_start(
                    out=wv_sb[0:1, 6, :], in_=wv_aug[F : F + 1, c0 : c0 + 390]
                )
                for b in range(BPC):
                    va = vaug_pool.tile([128, NKB, 390], bf16, tag="vaug")
                    for tb in range(NKB):
                        ps = ps_flex.tile([128, 512], f32, tag="flex", name="ps_v")
                        for c in range(6):
                            nc.tensor.matmul(
                                ps[:, 0:390],
                                xa[(b, c)][:, tb * 128 : (tb + 1) * 128],
                                wv_sb[:, c, :],
                                start=(c == 0),
                                stop=False,
                            )
                        nc.tensor.matmul(
                            ps[:, 0:390],
                            xa[(b, 6)][:, tb * 128 : (tb + 1) * 128],
                            wv_sb[0:1, 6, :],
                            start=False,
                            stop=True,
                        )
                        nc.scalar.activation(va[:, tb, :], ps[:, 0:390], AF.Copy)
                    vaug[(b, g)] = va
                    cs_col = csc_pool.tile([65, 6], f32, tag="cscol")
                    dma2.dma_start(out=cs_col, in_=cs_in[b, g])
                    csum[(b, g)] = cs_col

            # ---- main loop over head pairs ------------------------------
            for pair in range(6):
                g = pair // 3
                if pair % 3 == 0:
                    emit_vproj(g)

                # qT/kT projections for this pair, both batches
                wq_sb = wqk_pool.tile([128, 6, 128], bf16, tag="wq")
                wk_sb = wqk_pool.tile([128, 6, 128], bf16, tag="wk")
                p0 = pair * 128
                for c in range(6):
                    dma.dma_start(
                        out=wq_sb[:, c, :],
                        in_=wq_aug[c * 128 : (c + 1) * 128, p0 : p0 + 128],
                    )
                    dma.dma_start(
                        out=wk_sb[:, c, :],
                        in_=wk_aug[c * 128 : (c + 1) * 128, p0 : p0 + 128],
                    )
                bq_sb = bias_pool.tile([128, 2], f32, tag="bqk")
                dma.dma_start(out=bq_sb[:, 0:1], in_=bqk[0, p0 : p0 + 128])
                dma.dma_start(out=bq_sb[:, 1:2], in_=bqk[1, p0 : p0 + 128])

                qT = {}
                kT = {}
                for b in range(BPC):
                    qt = qkt_pool.tile([128, LP], bf16, tag="qT")
                    kt = qkt_pool.tile([128, LP], bf16, tag="kT")
                    for (dst, w_sb, scl, bi) in (
                        (qt, wq_sb, 0.125, 0),
                        (kt, wk_sb, 1.0, 1),
                    ):
                        for (q0, qw) in qsubs:
                            psq = ps_flex.tile(
                                [128, 512], f32, tag="flex", name="ps_qk"
                            )
                            for c in range(6):
                                nc.tensor.matmul(
                                    psq[:, 0:qw],
                                    w_sb[:, c, :],
                                    xa[(b, c)][:, q0 : q0 + qw],
                                    start=(c == 0), stop=(c == 5),
                                )
                            # relu(scale*xw + scale*b); the reference's +eps
                            # here is dropped (~1e-7 relative effect)
                            nc.scalar.activation(
                                dst[:, q0 : q0 + qw], psq[:, 0:qw], AF.Relu,
                                scale=scl, bias=bq_sb[:, bi : bi + 1],
                            )
                    qT[b] = qt
                    kT[b] = kt

                av_sbs = {}
                for hh in range(2):
                    h = pair * 2 + hh
                    r0 = hh * 64
                    # mask tiles for this head (shared across batches)
                    mks = []
                    for j in range(NKB):
                        mk = mask_pool.tile(
                            [128, QM], bf16, tag="mask", name="mask_tile"
                        )
                        dma.dma_start(out=mk, in_=mask_main[h, j])
                        mks.append(mk)
                    mkt = mtail_pool.tile([128, NKB], bf16, tag="mtail")
                    dma.dma_start(out=mkt, in_=mask_tail[h])

                    for b in range(BPC):
                        va = vaug[(b, pair // 3)]
                        vc0 = (pair % 3) * 130 + hh * 65

                        av = ps_av.tile([65, QM], f32, tag="ps_av")
                        ptl = ps_flex.tile(
                            [128, 512], f32, tag="flex", name="ps_tails"
                        )
                        stail = ptl[:, 0:NKB]
                        avt = ptl[0:65, NKB : NKB + 1]
                        mtt = mttail_pool.tile([128, NKB], bf16, tag="mttail")

                        # software-pipelined: AV_j issues after S_{j+1} so
                        # the PE never waits on the DVE mask-multiply
                        def emit_s(j):
                            lhs_k = kT[b][r0 : r0 + 64, j * 128 : (j + 1) * 128]
                            st = ps_s.tile([128, QM], f32, tag="ps_s")
                            for (q0, qw) in st_slices():
                                nc.tensor.matmul(
                                    st[:, q0 : q0 + qw],
                                    lhs_k,
                                    qT[b][r0 : r0 + 64, q0 : q0 + qw],
                                    start=True, stop=True,
                                )
                            # tail column q=1024 (shares the kT weights)
                            nc.tensor.matmul(
                                stail[:, j : j + 1],
                                lhs_k,
                                qT[b][r0 : r0 + 64, QM : QM + 1],
                                start=True, stop=True,
                            )
                            # masked scores -> bf16
                            mt = mt_pool.tile([128, QM], bf16, tag="mt")
                            nc.vector.tensor_mul(mt, st, mks[j])
                            return mt

                        def emit_av(j, mt):
                            # AV accumulation (row 64 = rowsum via ones col)
                            for (q0, qw) in st_slices():
                                nc.tensor.matmul(
                                    av[:, q0 : q0 + qw],
                                    va[:, j, vc0 : vc0 + 65],
                                    mt[:, q0 : q0 + qw],
                                    start=(j == 0), stop=(j == NKB - 1),
                                )

                        mt_prev = emit_s(0)
                        for j in range(1, NKB):
                            mt_j = emit_s(j)
                            emit_av(j - 1, mt_prev)
                            mt_prev = mt_j
                        emit_av(NKB - 1, mt_prev)

                        # tail: masked scores + AV
                        nc.vector.tensor_mul(mtt, stail, mkt)
                        for j in range(NKB):
                            nc.tensor.matmul(
                                avt,
                                va[:, j, vc0 : vc0 + 65],
                                mtt[:, j : j + 1],
                                start=(j == 0), stop=(j == NKB - 1),
                            )

                        # drain AV psum to SBUF (frees the banks for the
                        # next head while the normalize chain runs)
                        av_sb = avsb_pool.tile([65, L], f32, tag="avsb")
                        nc.scalar.activation(av_sb[:, 0:QM], av, AF.Copy)
                        nc.scalar.activation(av_sb[:, QM : QM + 1], avt, AF.Copy)
                        av_sbs[(hh, b)] = av_sb

                # ---- batched normalization for the pair's 4 (hh, b) -----
                # gather the 4 rowsum rows -> [4, L], one approx reciprocal
                rs4 = rs_pool.tile([4, L], f32, tag="rs")
                order = [(hh, b) for hh in range(2) for b in range(BPC)]
                for idx, (hh, b) in enumerate(order):
                    dma2.dma_start(
                        out=rs4[idx : idx + 1, :],
                        in_=av_sbs[(hh, b)][64:65, :],
                    )
                nc.vector.tensor_scalar_add(rs4, rs4, float(L) * EPS)
                rr4 = rs_pool.tile([4, L], f32, tag="rr")
                nc.vector.reciprocal(rr4, rs4)
                s0 = (pair % 2) * 4
                dma2.dma_start(out=rr_dram[s0 : s0 + 4, :], in_=rr4)
                for idx, (hh, b) in enumerate(order):
                    rr_slot = rr_dram[s0 + idx]
                    rr_bcast_src = bass.AP(
                        tensor=rr_slot.tensor,
                        offset=rr_slot.offset,
                        ap=[[0, 64]] + list(rr_slot.ap),
                    )
                    rrb = rrb_pool.tile([64, L], f32, tag="rrb")
                    dma2.dma_start(out=rrb, in_=rr_bcast_src)
                    hg = (pair % 3) * 2 + hh
                    cs = csum[(b, pair // 3)]
                    r0 = hh * 64
                    nc.vector.scalar_tensor_tensor(
                        ot_pairs[(b, pair)][r0 : r0 + 64, :],
                        av_sbs[(hh, b)][0:64, :],
                        cs[0:64, hg : hg + 1],
                        rrb,
                        op0=mybir.AluOpType.add,
                        op1=mybir.AluOpType.mult,
                    )

            # ---- output projection: yT = wo^T @ O^T + bo ----------------
            ctx.close()
            ctx = octx.enter_context(ExitStack())
            y_pool = ctx.enter_context(tc.tile_pool(name="y", bufs=7))
            ps_y = ctx.enter_context(tc.tile_pool(name="ps_y", bufs=2, space="PSUM"))

            oq_tiles = [(0, 512), (512, 512), (1024, 1)]
            for b in range(BPC):
                ys = []
                for fc in range(6):
                    ys.append(y_pool.tile([128, L], f32, tag="y", name="y_tile"))
                for (q0, qw) in oq_tiles:
                    for fc in range(6):
                        psy = ps_y.tile([128, 512], f32, tag="ps_y")
                        for hc in range(6):
                            nc.tensor.matmul(
                                psy[:, 0:qw],
                                wo_sb[hc][:, fc * 128 : (fc + 1) * 128],
                                ot_pairs[(b, hc)][:, q0 : q0 + qw],
                                start=(hc == 0), stop=(hc == 5),
                            )
                        # drain with bo fused as the per-partition bias
                        nc.scalar.activation(
                            ys[fc][:, q0 : q0 + qw], psy[:, 0:qw], AF.Identity,
                            bias=bo_sb[:, fc : fc + 1],
                        )
                for fc in range(6):
                    dma.dma_start(
                        out=yT[b, fc * 128 : (fc + 1) * 128, :], in_=ys[fc]
                    )

    _split_matmul_waits(nc)
    return nc


def _split_matmul_waits(nc):
    """Walrus TPB instruction structs encode a limited number of sync waits
    (the fp32 LDWEIGHTS+MATMUL pair can take none beyond its update).  Hoist
    excess waits onto same-engine NoOps inserted just before each
    instruction."""
    import bass_rust
    from concourse import mybir

    n = 0
    for f in nc.m.functions:
        for blk in f.blocks:
            insts = blk.instructions
            out = []
            for inst in insts:
                si = inst.sync_info
                tname = type(inst).__name__
                if si is not None and len(si.on_wait) > 0 and tname != "InstISA":
                    # custom-DVE (InstCustomDveAnt) encodes no extra wait
                    # slots either
                    cap = 0 if tname in ("InstMatmult", "InstCustomDveAnt") else 1
                    waits = list(si.on_wait)
                    if len(waits) > cap:
                        hoist = waits[: len(waits) - cap]
                        keep = waits[len(waits) - cap :]
                        for w in hoist:
                            nop = mybir.InstNoOp(
                                name=f"I-mmw-{n}", ins=[], outs=[]
                            )
                            n += 1
                            nop.engine = inst.engine
                            nop.sync_info = bass_rust.SyncInfo(
                                on_wait=[w], on_update=[]
                            )
                            out.append(nop)
                        inst.sync_info = bass_rust.SyncInfo(
                            on_wait=keep, on_update=list(si.on_update)
                        )
                out.append(inst)
            insts[:] = out
    return n


def _dist_index():
    gi = np.arange(NB)
    gj = np.arange(NB)
    idx = (
        (gi[:, None, None, None] - gi[None, None, :, None] + NB) * 2 * NB
        + gj[None, :, None, None]
        - gj[None, None, None, :]
        + NB
    )
    return idx.reshape(-1).astype(np.int32)


def _host_prep(x, wq, bq, wk, bk, wv, bv, wo, bo, toeplitz_params):
    import ml_dtypes

    f4 = np.float32
    bf = ml_dtypes.bfloat16
    x = np.asarray(x, f4)
    L0 = NB * NB

    xaT = np.zeros((B, FA, LP), bf)
    xaT[:, :F, :L] = np.transpose(x, (0, 2, 1)).astype(bf)
    xaT[:, F, :L] = 1.0

    wq_aug = np.empty((FA, F), bf)
    wq_aug[:F] = np.asarray(wq, f4).reshape(F, F).astype(bf)
    wq_aug[F] = np.asarray(bq, f4).reshape(F).astype(bf)
    wk_aug = np.empty((FA, F), bf)
    wk_aug[:F] = np.asarray(wk, f4).reshape(F, F).astype(bf)
    wk_aug[F] = np.asarray(bk, f4).reshape(F).astype(bf)

    wv_aug = np.zeros((FA, H * 65), bf)
    wvr = np.asarray(wv, f4)
    bvr = np.asarray(bv, f4)
    for h in range(H):
        wv_aug[:F, h * 65 : h * 65 + 64] = wvr[:, h, :].astype(bf)
        wv_aug[F, h * 65 : h * 65 + 64] = bvr[h].astype(bf)
        wv_aug[F, h * 65 + 64] = 1.0

    wo_flat = np.ascontiguousarray(
        np.asarray(wo, f4).reshape(H * D, F).astype(bf)
    )
    bo_arr = np.asarray(bo, f4).reshape(F)

    # gathered |toeplitz| mask, padded (CLS row/col of ones), transposed,
    # k padded to 1152 with zeros
    tp = np.asarray(toeplitz_params, f4)
    tm = np.abs(tp[:, _dist_index()]).reshape(H, L0, L0)
    tm_full = np.ones((H, L, L), f4)
    tm_full[:, 1:, 1:] = tm
    maskT = np.zeros((H, LP, L), bf)
    maskT[:, :L, :] = np.transpose(tm_full, (0, 2, 1)).astype(bf)
    maskT_main = np.ascontiguousarray(
        maskT[:, :, :QM].reshape(H, NKB, 128, QM)
    )
    maskT_tail = np.ascontiguousarray(
        maskT[:, :, QM].reshape(H, NKB, 128).transpose(0, 2, 1)
    )

    xsum = x[:, :, :].sum(axis=1)  # [B, F]
    cs = np.einsum("bf,fhd->bhd", xsum, wvr) + L * bvr[None]  # [B, H, 64]
    cs_full = np.concatenate(
        [cs, np.full((B, H, 1), float(L), np.float32)], axis=2
    ) * np.float32(EPS)  # [B, H, 65]
    cs_cols = np.zeros((B, 2, 65, 6), f4)
    for g in range(2):
        for hh in range(6):
            cs_cols[:, g, :, hh] = cs_full[:, 6 * g + hh, :]
    bqk_eff = np.stack(
        [np.asarray(bq, f4).reshape(F) * 0.125, np.asarray(bk, f4).reshape(F)]
    )
    shared = dict(
        bqk_eff=bqk_eff,
        wq_aug=wq_aug,
        wk_aug=wk_aug,
        wv_aug=wv_aug,
        wo_flat=wo_flat,
        bo=bo_arr,
        maskT_main=maskT_main,
        maskT_tail=maskT_tail,
    )
    in_maps = []
    for c in range(NCORES):
        m = dict(shared)
        m["xaT"] = np.ascontiguousarray(xaT[c * BPC : (c + 1) * BPC])
        m["cs_cols"] = np.ascontiguousarray(cs_cols[c * BPC : (c + 1) * BPC])
        in_maps.append(m)
    return in_maps


def _get_program():
    global _PROG
    if _PROG is None:
        _PROG = _build_program()
    return _PROG


def run(trace=False, **inputs):
    from concourse.bass_utils import run_bass_kernel_spmd

    nc = _get_program()
    in_maps = _host_prep(**inputs)
    res = run_bass_kernel_spmd(nc, in_maps, list(range(NCORES)), trace=trace)
    outs = []
    for c in range(NCORES):
        yt = res.results[c]["yT"]  # [BPC, F, L]
        outs.append(np.transpose(yt, (0, 2, 1)))
    y = np.concatenate(outs, axis=0).astype(np.float32)
    return y, res


def kernel(**inputs):
    y, _ = run(trace=False, **inputs)
    return y


# revision 25
# speedup vs baseline: 1.1223x; 1.0009x over previous
"""Trainium2 Bass kernel for relu-kernelized multi-head attention with a
per-head Toeplitz relative-position mask (sparse_attention problem).

Contract: kernel(**inputs) takes FULL unsharded inputs (numpy), returns the
FULL output [16, 1025, 768]. Internally: data-parallel over batch across 8
NeuronCores (2 batches/core), identical SPMD program, per-core inputs differ
only in the x shard.

Math (per batch b):
  q = relu((x@wq + bq)/8) + eps ; k = relu(x@wk + bk) + eps ; v = x@wv + bv
  S[q,k] = sum_d q*k ;  attn = S*|tm| + eps ; attn /= rowsum ; out = attn@v
  y = out@wo + bo

v2: all matmul operands in bf16 (PE runs 1 cycle/row vs fp32's 4; mask DMA
halves).  PSUM accumulation stays fp32.  The q/k "+eps" is dropped (its
effect is ~1e-7 relative; the attention-level eps is kept exactly via the
cs_cols rank-1 correction and the rowsum + L*eps denominator).  The row
normalization is batched per head-pair: 4 rowsum rows are gathered into one
[4, L] tile, one reciprocal_approx_fast (~51 ULP, fine vs the 2e-2 gate)
replaces 4 serial [1, L] full-precision reciprocals.  Attention outputs stay
resident in SBUF as 12 [128, L] bf16 head-pair tiles consumed directly by
the output projection (no DRAM spill), with bo fused into the drain
activation.

Device-side layout choices:
  - x shipped transposed+padded with a ones-row: xaT [2, 769, 1152] so the
    V bias folds into the matmul as a K=1 extra contraction chunk.
  - qT/kT produced in [head*64, token] layout -> S^T tiles [k,q] come
    straight from matmuls with K=d=64.
  - mask |tm| is gathered on host (pure input preprocessing: a
    Toeplitz-strided view of toeplitz_params), shipped transposed
    [h, k, q] in bf16, padded with zeros on the k dim.
  - v_aug [token, 65] per head carries a ones column: the AV matmul's row 64
    accumulates the rowsum for free.  The "+eps" of the reference rides in
    as a rank-1 correction: eps * colsum(v_aug), added during normalization.
"""

import os
import sys

sys.path.insert(0, "/opt/trn_rl_repo")

import numpy as np

B, L, F, H, D = 16, 1025, 768, 12, 64
NB = 32
EPS = 1e-8
LP = 1152           # padded token count (9 * 128)
NKB = 9             # k blocks of 128
QM = 1024           # main q width (q tail = 1 col, index 1024)
FA = F + 1          # augmented contraction (ones row)
NCORES = 8
BPC = B // NCORES   # batches per core

_PROG = None


def _build_program():
    import concourse.bass as bass
    import concourse.tile as tile
    from concourse import mybir

    f32 = mybir.dt.float32
    bf16 = mybir.dt.bfloat16
    AF = mybir.ActivationFunctionType

    nc = bass.Bass()

    xaT = nc.declare_dram_parameter("xaT", [BPC, FA, LP], bf16, isOutput=False)
    wq_aug = nc.declare_dram_parameter("wq_aug", [FA, F], bf16, isOutput=False)
    wk_aug = nc.declare_dram_parameter("wk_aug", [FA, F], bf16, isOutput=False)
    wv_aug = nc.declare_dram_parameter("wv_aug", [FA, H * 65], bf16, isOutput=False)
    wo_flat = nc.declare_dram_parameter("wo_flat", [H * D, F], bf16, isOutput=False)
    bo_in = nc.declare_dram_parameter("bo", [F], f32, isOutput=False)
    mask_main = nc.declare_dram_parameter(
        "maskT_main", [H, NKB, 128, QM], bf16, isOutput=False
    )
    mask_tail = nc.declare_dram_parameter(
        "maskT_tail", [H, 128, NKB], bf16, isOutput=False
    )
    yT = nc.declare_dram_parameter("yT", [BPC, F, L], f32, isOutput=True)

    rr_dram = nc.dram_tensor("rr_dram", [8, L], f32)
    bqk = nc.declare_dram_parameter("bqk_eff", [2, F], f32, isOutput=False)
    cs_in = nc.declare_dram_parameter("cs_cols", [BPC, 2, 65, 6], f32, isOutput=False)

    with tile.TileContext(nc) as tc:
        from contextlib import ExitStack

        with ExitStack() as octx:
            consts = octx.enter_context(tc.tile_pool(name="consts", bufs=1))
            # attention outputs, SBUF-resident across phases: 12 tiles
            # [128, L] bf16, one per (batch, head-pair); rows 0:64 = even
            # head, 64:128 = odd head of the pair
            ot_pool = octx.enter_context(tc.tile_pool(name="ot", bufs=2 * 6))
            wo_pool = octx.enter_context(tc.tile_pool(name="wo", bufs=6))
            bo_pool = octx.enter_context(tc.tile_pool(name="bo", bufs=1))
            ctx = octx.enter_context(ExitStack())
            xa_pool = ctx.enter_context(tc.tile_pool(name="xa", bufs=2 * 6))
            wqk_pool = ctx.enter_context(tc.tile_pool(name="wqk", bufs=2))
            wv_pool = ctx.enter_context(tc.tile_pool(name="wv", bufs=2))
            qkt_pool = ctx.enter_context(tc.tile_pool(name="qkt", bufs=2))
            vaug_pool = ctx.enter_context(tc.tile_pool(name="vaug", bufs=4))
            csc_pool = ctx.enter_context(tc.tile_pool(name="cscol", bufs=2))
            bias_pool = ctx.enter_context(tc.tile_pool(name="bias", bufs=2))
            mask_pool = ctx.enter_context(tc.tile_pool(name="mask", bufs=12))
            mtail_pool = ctx.enter_context(tc.tile_pool(name="mtail", bufs=2))
            mt_pool = ctx.enter_context(tc.tile_pool(name="mt", bufs=3))
            mttail_pool = ctx.enter_context(tc.tile_pool(name="mttail", bufs=2))
            rs_pool = ctx.enter_context(tc.tile_pool(name="rs", bufs=1))
            rrb_pool = ctx.enter_context(tc.tile_pool(name="rrb", bufs=4))
            avsb_pool = ctx.enter_context(tc.tile_pool(name="avsb", bufs=5))

            # flex pool: [128,512] tiles time-shared between projection psums
            # (2-deep so the activation drain doesn't stall the next matmul
            # group) and the per-head tail psum (stail+avt live in a slice)
            ps_flex = ctx.enter_context(
                tc.tile_pool(name="ps_flex", bufs=2, space="PSUM")
            )
            ps_s = ctx.enter_context(tc.tile_pool(name="ps_s", bufs=2, space="PSUM"))
            ps_av = ctx.enter_context(tc.tile_pool(name="ps_av", bufs=1, space="PSUM"))

            dma = nc.sync
            dma2 = nc.gpsimd  # second DMA-issue queue for the normalize path

            # constants
            ones_row = consts.tile([1, LP], bf16)
            nc.vector.memset(ones_row[:, 0:L], 1.0)
            nc.vector.memset(ones_row[:, L:LP], 0.0)

            ot_pairs = {}
            for b in range(BPC):
                for pair in range(6):
                    ot_pairs[(b, pair)] = ot_pool.tile(
                        [128, L], bf16, tag="ot", name="ot_pair"
                    )

            # ---- persistent xaT in SBUF --------------------------------
            # 6 full 128-row chunks per batch + the ones-row (row 768)
            xa = {}
            for b in range(BPC):
                for c in range(6):
                    t = xa_pool.tile([128, LP], bf16, tag="xa", name="xa_tile")
                    dma.dma_start(out=t, in_=xaT[b, c * 128 : (c + 1) * 128, :])
                    xa[(b, c)] = t
            for b in range(BPC):
                xa[(b, 6)] = ones_row

            # output-projection weights, prefetched so the O phase starts
            # without a DMA stall
            bo_sb = bo_pool.tile([128, 6], f32)
            for fc in range(6):
                dma.dma_start(
                    out=bo_sb[:, fc : fc + 1], in_=bo_in[fc * 128 : (fc + 1) * 128]
                )
            wo_sb = []
            for hc in range(6):
                t = wo_pool.tile([128, F], bf16, tag="wo", name="wo_tile")
                dma.dma_start(out=t, in_=wo_flat[hc * 128 : (hc + 1) * 128, :])
                wo_sb.append(t)

            # q sub-tiles for projections (moving dim <= 512); only token
            # 1024 of the padded tail is real
            qsubs = [(0, 512), (512, 512), (1024, 1)]
            # attention q tiling: main [0,1024) in 2 psum-bank halves + tail col
            def st_slices():
                return [(0, 512), (512, 512)]

            # ---- v projections + colsums, per 3-pair group --------------
            # wv_aug columns are grouped per head: h*65 + (0..63 -> wv, 64 -> ones)
            vaug = {}      # (b, g) -> [128, NKB, 390]
            csum = {}      # (b, g) -> [65, 6]

            def emit_vproj(g):
                wv_sb = wv_pool.tile([128, 7, 390], bf16, tag="wv")
                c0 = g * 390
                for c in range(6):
                    dma.dma_start(
                        out=wv_sb[:, c, :],
                        in_=wv_aug[c * 128 : (c + 1) * 128, c0 : c0 + 390],
                    )
                dma.dma_start(
                    out=wv_sb[0:1, 6, :], in_=wv_aug[F : F + 1, c0 : c0 + 390]
                )
                for b in range(BPC):
                    va = vaug_pool.tile([128, NKB, 390], bf16, tag="vaug")
                    for tb in range(NKB):
                        ps = ps_flex.tile([128, 512], f32, tag="flex", name="ps_v")
                        for c in range(6):
                            nc.tensor.matmul(
                                ps[:, 0:390],
                                xa[(b, c)][:, tb * 128 : (tb + 1) * 128],
                                wv_sb[:, c, :],
                                start=(c == 0),
                                stop=False,
                            )
                        nc.tensor.matmul(
                            ps[:, 0:390],
                            xa[(b, 6)][:, tb * 128 : (tb + 1) * 128],
                            wv_sb[0:1, 6, :],
                            start=False,
                            stop=True,
                        )
                        nc.scalar.activation(va[:, tb, :], ps[:, 0:390], AF.Copy)
                    vaug[(b, g)] = va
                    cs_col = csc_pool.tile([65, 6], f32, tag="cscol")
                    dma2.dma_start(out=cs_col, in_=cs_in[b, g])
                    csum[(b, g)] = cs_col

            # ---- main loop over head pairs ------------------------------
            for pair in range(6):
                g = pair // 3
                if pair % 3 == 0:
                    emit_vproj(g)

                # qT/kT projections for this pair, both batches
                wq_sb = wqk_pool.tile([128, 6, 128], bf16, tag="wq")
                wk_sb = wqk_pool.tile([128, 6, 128], bf16, tag="wk")
                p0 = pair * 128
                for c in range(6):
                    dma.dma_start(
                        out=wq_sb[:, c, :],
                        in_=wq_aug[c * 128 : (c + 1) * 128, p0 : p0 + 128],
                    )
                    dma.dma_start(
                        out=wk_sb[:, c, :],
                        in_=wk_aug[c * 128 : (c + 1) * 128, p0 : p0 + 128],
                    )
                bq_sb = bias_pool.tile([128, 2], f32, tag="bqk")
                dma.dma_start(out=bq_sb[:, 0:1], in_=bqk[0, p0 : p0 + 128])
                dma.dma_start(out=bq_sb[:, 1:2], in_=bqk[1, p0 : p0 + 128])

                qT = {}
                kT = {}
                for b in range(BPC):
                    qt = qkt_pool.tile([128, LP], bf16, tag="qT")
                    kt = qkt_pool.tile([128, LP], bf16, tag="kT")
                    # k-pad columns are read by the j=8 S matmul (masked to
                    # zero afterwards) - keep them finite
                    nc.vector.memset(kt[:, L:LP], 0.0)
                    for (dst, w_sb, scl, bi) in (
                        (qt, wq_sb, 0.125, 0),
                        (kt, wk_sb, 1.0, 1),
                    ):
                        for (q0, qw) in qsubs:
                            psq = ps_flex.tile(
                                [128, 512], f32, tag="flex", name="ps_qk"
                            )
                            for c in range(6):
                                nc.tensor.matmul(
                                    psq[:, 0:qw],
                                    w_sb[:, c, :],
                                    xa[(b, c)][:, q0 : q0 + qw],
                                    start=(c == 0), stop=(c == 5),
                                )
                            # relu(scale*xw + scale*b); the reference's +eps
                            # here is dropped (~1e-7 relative effect)
                            nc.scalar.activation(
                                dst[:, q0 : q0 + qw], psq[:, 0:qw], AF.Relu,
                                scale=scl, bias=bq_sb[:, bi : bi + 1],
                            )
                    qT[b] = qt
                    kT[b] = kt

                av_sbs = {}
                for hh in range(2):
                    h = pair * 2 + hh
                    r0 = hh * 64
                    # mask tiles for this head (shared across batches)
                    mks = []
                    for j in range(NKB):
                        mk = mask_pool.tile(
                            [128, QM], bf16, tag="mask", name="mask_tile"
                        )
                        dma.dma_start(out=mk, in_=mask_main[h, j])
                        mks.append(mk)
                    mkt = mtail_pool.tile([128, NKB], bf16, tag="mtail")
                    dma.dma_start(out=mkt, in_=mask_tail[h])

                    for b in range(BPC):
                        va = vaug[(b, pair // 3)]
                        vc0 = (pair % 3) * 130 + hh * 65

                        av = ps_av.tile([65, QM], f32, tag="ps_av")
                        ptl = ps_flex.tile(
                            [128, 512], f32, tag="flex", name="ps_tails"
                        )
                        stail = ptl[:, 0:NKB]
                        avt = ptl[0:65, NKB : NKB + 1]
                        mtt = mttail_pool.tile([128, NKB], bf16, tag="mttail")

                        # software-pipelined: AV_j issues after S_{j+1} so
                        # the PE never waits on the DVE mask-multiply
                        def emit_s(j):
                            lhs_k = kT[b][r0 : r0 + 64, j * 128 : (j + 1) * 128]
                            st = ps_s.tile([128, QM], f32, tag="ps_s")
                            for (q0, qw) in st_slices():
                                nc.tensor.matmul(
                                    st[:, q0 : q0 + qw],
                                    lhs_k,
                                    qT[b][r0 : r0 + 64, q0 : q0 + qw],
                                    start=True, stop=True,
                                )
                            # tail column q=1024 (shares the kT weights)
                            nc.tensor.matmul(
                                stail[:, j : j + 1],
                                lhs_k,
                                qT[b][r0 : r0 + 64, QM : QM + 1],
                                start=True, stop=True,
                            )
                            # masked scores -> bf16
                            mt = mt_pool.tile([128, QM], bf16, tag="mt")
                            nc.vector.tensor_mul(mt, st, mks[j])
                            return mt

                        def emit_av(j, mt):
                            # AV accumulation (row 64 = rowsum via ones col)
                            for (q0, qw) in st_slices():
                                nc.tensor.matmul(
                                    av[:, q0 : q0 + qw],
                                    va[:, j, vc0 : vc0 + 65],
                                    mt[:, q0 : q0 + qw],
                                    start=(j == 0), stop=(j == NKB - 1),
                                )

                        mt_prev = emit_s(0)
                        for j in range(1, NKB):
                            mt_j = emit_s(j)
                            emit_av(j - 1, mt_prev)
                            mt_prev = mt_j
                        emit_av(NKB - 1, mt_prev)

                        # tail: masked scores + AV
                        nc.vector.tensor_mul(mtt, stail, mkt)
                        for j in range(NKB):
                            nc.tensor.matmul(
                                avt,
                                va[:, j, vc0 : vc0 + 65],
                                mtt[:, j : j + 1],
                                start=(j == 0), stop=(j == NKB - 1),
                            )

                        # drain AV psum to SBUF (frees the banks for the
                        # next head while the normalize chain runs)
                        av_sb = avsb_pool.tile([65, L], f32, tag="avsb")
                        nc.scalar.activation(av_sb[:, 0:QM], av, AF.Copy)
                        nc.scalar.activation(av_sb[:, QM : QM + 1], avt, AF.Copy)
                        av_sbs[(hh, b)] = av_sb

                # ---- batched normalization for the pair's 4 (hh, b) -----
                # gather rowsum rows into one tile, one batched reciprocal
                def normalize(combos, slot0):
                    n = len(combos)
                    rs = rs_pool.tile([4, L], f32, tag="rs")
                    for idx, (hh, b) in enumerate(combos):
                        dma2.dma_start(
                            out=rs[idx : idx + 1, :],
                            in_=av_sbs[(hh, b)][64:65, :],
                        )
                    nc.vector.tensor_scalar_add(
                        rs[0:n], rs[0:n], float(L) * EPS
                    )
                    rr = rs_pool.tile([4, L], f32, tag="rr")
                    nc.vector.reciprocal(rr[0:n], rs[0:n])
                    dma2.dma_start(out=rr_dram[slot0 : slot0 + n, :], in_=rr[0:n])
                    for idx, (hh, b) in enumerate(combos):
                        rr_slot = rr_dram[slot0 + idx]
                        rr_bcast_src = bass.AP(
                            tensor=rr_slot.tensor,
                            offset=rr_slot.offset,
                            ap=[[0, 64]] + list(rr_slot.ap),
                        )
                        rrb = rrb_pool.tile([64, L], f32, tag="rrb")
                        dma2.dma_start(out=rrb, in_=rr_bcast_src)
                        hg = (pair % 3) * 2 + hh
                        cs = csum[(b, pair // 3)]
                        r0h = hh * 64
                        nc.vector.scalar_tensor_tensor(
                            ot_pairs[(b, pair)][r0h : r0h + 64, :],
                            av_sbs[(hh, b)][0:64, :],
                            cs[0:64, hg : hg + 1],
                            rrb,
                            op0=mybir.AluOpType.add,
                            op1=mybir.AluOpType.mult,
                        )

                if pair < 5:
                    normalize(
                        [(hh, b) for hh in range(2) for b in range(BPC)],
                        (pair % 2) * 4,
                    )
                else:
                    # last pair: per-batch so the O projection of b=0 isn't
                    # gated on b=1's normalize chain
                    normalize([(0, 0), (1, 0)], 4)
                    normalize([(0, 1), (1, 1)], 6)

            # ---- output projection: yT = wo^T @ O^T + bo ----------------
            ctx.close()
            ctx = octx.enter_context(ExitStack())
            y_pool = ctx.enter_context(tc.tile_pool(name="y", bufs=7))
            ps_y = ctx.enter_context(tc.tile_pool(name="ps_y", bufs=2, space="PSUM"))

            oq_tiles = [(0, 512), (512, 512), (1024, 1)]
            for b in range(BPC):
                ys = []
                for fc in range(6):
                    ys.append(y_pool.tile([128, L], f32, tag="y", name="y_tile"))
                for (q0, qw) in oq_tiles:
                    for fc in range(6):
                        psy = ps_y.tile([128, 512], f32, tag="ps_y")
                        for hc in range(6):
                            nc.tensor.matmul(
                                psy[:, 0:qw],
                                wo_sb[hc][:, fc * 128 : (fc + 1) * 128],
                                ot_pairs[(b, hc)][:, q0 : q0 + qw],
                                start=(hc == 0), stop=(hc == 5),
                            )
                        # drain with bo fused as the per-partition bias
                        nc.scalar.activation(
                            ys[fc][:, q0 : q0 + qw], psy[:, 0:qw], AF.Identity,
                            bias=bo_sb[:, fc : fc + 1],
                        )
                for fc in range(6):
                    dma.dma_start(
                        out=yT[b, fc * 128 : (fc + 1) * 128, :], in_=ys[fc]
                    )

    _split_matmul_waits(nc)
    return nc


def _split_matmul_waits(nc):
    """Walrus TPB instruction structs encode a limited number of sync waits
    (the fp32 LDWEIGHTS+MATMUL pair can take none beyond its update).  Hoist
    excess waits onto same-engine NoOps inserted just before each
    instruction."""
    import bass_rust
    from concourse import mybir

    n = 0
    for f in nc.m.functions:
        for blk in f.blocks:
            insts = blk.instructions
            out = []
            for inst in insts:
                si = inst.sync_info
                tname = type(inst).__name__
                if si is not None and len(si.on_wait) > 0 and tname != "InstISA":
                    # custom-DVE (InstCustomDveAnt) encodes no extra wait
                    # slots either
                    cap = 0 if tname in ("InstMatmult", "InstCustomDveAnt") else 1
                    waits = list(si.on_wait)
                    if len(waits) > cap:
                        hoist = waits[: len(waits) - cap]
                        keep = waits[len(waits) - cap :]
                        for w in hoist:
                            nop = mybir.InstNoOp(
                                name=f"I-mmw-{n}", ins=[], outs=[]
                            )
                            n += 1
                            nop.engine = inst.engine
                            nop.sync_info = bass_rust.SyncInfo(
                                on_wait=[w], on_update=[]
                            )
                            out.append(nop)
                        inst.sync_info = bass_rust.SyncInfo(
                            on_wait=keep, on_update=list(si.on_update)
                        )
                out.append(inst)
            insts[:] = out
    return n


def _dist_index():
    gi = np.arange(NB)
    gj = np.arange(NB)
    idx = (
        (gi[:, None, None, None] - gi[None, None, :, None] + NB) * 2 * NB
        + gj[None, :, None, None]
        - gj[None, None, None, :]
        + NB
    )
    return idx.reshape(-1).astype(np.int32)


def _host_prep(x, wq, bq, wk, bk, wv, bv, wo, bo, toeplitz_params):
    import ml_dtypes

    f4 = np.float32
    bf = ml_dtypes.bfloat16
    x = np.asarray(x, f4)
    L0 = NB * NB

    xaT = np.zeros((B, FA, LP), bf)
    xaT[:, :F, :L] = np.transpose(x, (0, 2, 1)).astype(bf)
    xaT[:, F, :L] = 1.0

    wq_aug = np.empty((FA, F), bf)
    wq_aug[:F] = np.asarray(wq, f4).reshape(F, F).astype(bf)
    wq_aug[F] = np.asarray(bq, f4).reshape(F).astype(bf)
    wk_aug = np.empty((FA, F), bf)
    wk_aug[:F] = np.asarray(wk, f4).reshape(F, F).astype(bf)
    wk_aug[F] = np.asarray(bk, f4).reshape(F).astype(bf)

    wv_aug = np.zeros((FA, H * 65), bf)
    wvr = np.asarray(wv, f4)
    bvr = np.asarray(bv, f4)
    for h in range(H):
        wv_aug[:F, h * 65 : h * 65 + 64] = wvr[:, h, :].astype(bf)
        wv_aug[F, h * 65 : h * 65 + 64] = bvr[h].astype(bf)
        wv_aug[F, h * 65 + 64] = 1.0

    wo_flat = np.ascontiguousarray(
        np.asarray(wo, f4).reshape(H * D, F).astype(bf)
    )
    bo_arr = np.asarray(bo, f4).reshape(F)

    # gathered |toeplitz| mask, padded (CLS row/col of ones), transposed,
    # k padded to 1152 with zeros
    tp = np.asarray(toeplitz_params, f4)
    tm = np.abs(tp[:, _dist_index()]).reshape(H, L0, L0)
    tm_full = np.ones((H, L, L), f4)
    tm_full[:, 1:, 1:] = tm
    maskT = np.zeros((H, LP, L), bf)
    maskT[:, :L, :] = np.transpose(tm_full, (0, 2, 1)).astype(bf)
    maskT_main = np.ascontiguousarray(
        maskT[:, :, :QM].reshape(H, NKB, 128, QM)
    )
    maskT_tail = np.ascontiguousarray(
        maskT[:, :, QM].reshape(H, NKB, 128).transpose(0, 2, 1)
    )

    xsum = x[:, :, :].sum(axis=1)  # [B, F]
    cs = np.einsum("bf,fhd->bhd", xsum, wvr) + L * bvr[None]  # [B, H, 64]
    cs_full = np.concatenate(
        [cs, np.full((B, H, 1), float(L), np.float32)], axis=2
    ) * np.float32(EPS)  # [B, H, 65]
    cs_cols = np.zeros((B, 2, 65, 6), f4)
    for g in range(2):
        for hh in range(6):
            cs_cols[:, g, :, hh] = cs_full[:, 6 * g + hh, :]
    bqk_eff = np.stack(
        [np.asarray(bq, f4).reshape(F) * 0.125, np.asarray(bk, f4).reshape(F)]
    )
    shared = dict(
        bqk_eff=bqk_eff,
        wq_aug=wq_aug,
        wk_aug=wk_aug,
        wv_aug=wv_aug,
        wo_flat=wo_flat,
        bo=bo_arr,
        maskT_main=maskT_main,
        maskT_tail=maskT_tail,
    )
    in_maps = []
    for c in range(NCORES):
        m = dict(shared)
        m["xaT"] = np.ascontiguousarray(xaT[c * BPC : (c + 1) * BPC])
        m["cs_cols"] = np.ascontiguousarray(cs_cols[c * BPC : (c + 1) * BPC])
        in_maps.append(m)
    return in_maps


def _get_program():
    global _PROG
    if _PROG is None:
        _PROG = _build_program()
    return _PROG


def run(trace=False, **inputs):
    from concourse.bass_utils import run_bass_kernel_spmd

    nc = _get_program()
    in_maps = _host_prep(**inputs)
    res = run_bass_kernel_spmd(nc, in_maps, list(range(NCORES)), trace=trace)
    outs = []
    for c in range(NCORES):
        yt = res.results[c]["yT"]  # [BPC, F, L]
        outs.append(np.transpose(yt, (0, 2, 1)))
    y = np.concatenate(outs, axis=0).astype(np.float32)
    return y, res


def kernel(**inputs):
    y, _ = run(trace=False, **inputs)
    return y


# revision 28
# speedup vs baseline: 1.1439x; 1.0192x over previous
"""Trainium2 Bass kernel for relu-kernelized multi-head attention with a
per-head Toeplitz relative-position mask (sparse_attention problem).

Contract: kernel(**inputs) takes FULL unsharded inputs (numpy), returns the
FULL output [16, 1025, 768]. Internally: data-parallel over batch across 8
NeuronCores (2 batches/core), identical SPMD program, per-core inputs differ
only in the x shard.

Math (per batch b):
  q = relu((x@wq + bq)/8) + eps ; k = relu(x@wk + bk) + eps ; v = x@wv + bv
  S[q,k] = sum_d q*k ;  attn = S*|tm| + eps ; attn /= rowsum ; out = attn@v
  y = out@wo + bo

Perf structure (v6):
  - all matmul operands bf16 (PE 1 cycle/row vs fp32's 4), fp32 PSUM.
  - every logical load is ONE DMA: host pre-packs all tensors in the exact
    [partition, ...] SBUF layout (DMA issue on the sync queue costs ~650ns
    each - the v2 kernel spent >160us there).
  - S/AV j-loop is software-pipelined (AV_j after S_{j+1}) so the PE never
    waits on the DVE mask-multiply.
  - row-normalization batched per head pair: one [4,L] reciprocal, DMA
    partition-broadcast of 1/r via a DRAM bounce, all on the gpsimd queue.
  - attention outputs stay in SBUF as 12 [128,L] bf16 head-pair tiles
    consumed directly by the O projection; output shipped bf16.
  - the q/k "+eps" of the reference is dropped (~1e-7 relative effect); the
    attention-level eps is kept via the cs rank-1 correction and the
    rowsum + L*eps denominator.
"""

import os
import sys

sys.path.insert(0, "/opt/trn_rl_repo")

import numpy as np

B, L, F, H, D = 16, 1025, 768, 12, 64
NB = 32
EPS = 1e-8
LP = 1152           # padded token count (9 * 128)
NKB = 9             # k blocks of 128
QM = 1024           # main q width (q tail = 1 col, index 1024)
FA = F + 1          # augmented contraction (ones row)
NCORES = 8
BPC = B // NCORES   # batches per core

_PROG = None


def _build_program():
    import concourse.bass as bass
    import concourse.tile as tile
    from concourse import mybir

    f32 = mybir.dt.float32
    bf16 = mybir.dt.bfloat16
    AF = mybir.ActivationFunctionType

    nc = bass.Bass()

    xaP = nc.declare_dram_parameter("xaP", [BPC, 128, 6, LP], bf16, isOutput=False)
    wqP = nc.declare_dram_parameter("wqP", [6, 128, 6, 128], bf16, isOutput=False)
    wkP = nc.declare_dram_parameter("wkP", [6, 128, 6, 128], bf16, isOutput=False)
    wvP = nc.declare_dram_parameter("wvP", [2, 128, 7, 390], bf16, isOutput=False)
    woP = nc.declare_dram_parameter("woP", [128, 6, F], bf16, isOutput=False)
    boP = nc.declare_dram_parameter("boP", [128, 6], f32, isOutput=False)
    bqkP = nc.declare_dram_parameter("bqkP", [128, 12], f32, isOutput=False)
    csP = nc.declare_dram_parameter("csP", [65, 24], f32, isOutput=False)
    maskP = nc.declare_dram_parameter(
        "maskP", [H, 128, NKB, QM], bf16, isOutput=False
    )
    mask_tail = nc.declare_dram_parameter(
        "maskT_tail", [H, 128, NKB], bf16, isOutput=False
    )
    yT = nc.declare_dram_parameter("yT", [BPC, 128, 6, L], bf16, isOutput=True)

    rr_dram = nc.dram_tensor("rr_dram", [8, L], f32)

    with tile.TileContext(nc) as tc:
        from contextlib import ExitStack

        with ExitStack() as octx:
            consts = octx.enter_context(tc.tile_pool(name="consts", bufs=1))
            # attention outputs, SBUF-resident across phases: 12 tiles
            # [128, L] bf16, one per (batch, head-pair); rows 0:64 = even
            # head, 64:128 = odd head of the pair
            ot_pool = octx.enter_context(tc.tile_pool(name="ot", bufs=2 * 6))
            wo_pool = octx.enter_context(tc.tile_pool(name="wo", bufs=1))
            ctx = octx.enter_context(ExitStack())
            xa_pool = ctx.enter_context(tc.tile_pool(name="xa", bufs=2))
            wqk_pool = ctx.enter_context(tc.tile_pool(name="wqk", bufs=2))
            wv_pool = ctx.enter_context(tc.tile_pool(name="wv", bufs=2))
            qkt_pool = ctx.enter_context(tc.tile_pool(name="qkt", bufs=2))
            vaug_pool = ctx.enter_context(tc.tile_pool(name="vaug", bufs=4))
            mask_pool = ctx.enter_context(tc.tile_pool(name="mask", bufs=2))
            mtail_pool = ctx.enter_context(tc.tile_pool(name="mtail", bufs=2))
            mt_pool = ctx.enter_context(tc.tile_pool(name="mt", bufs=3))
            mttail_pool = ctx.enter_context(tc.tile_pool(name="mttail", bufs=2))
            rs_pool = ctx.enter_context(tc.tile_pool(name="rs", bufs=1))
            rrb_pool = ctx.enter_context(tc.tile_pool(name="rrb", bufs=3))
            avsb_pool = ctx.enter_context(tc.tile_pool(name="avsb", bufs=4))

            # flex pool: [128,512] tiles time-shared between projection psums
            # (2-deep so the activation drain doesn't stall the next matmul
            # group) and the per-head tail psum (stail+avt live in a slice)
            ps_flex = ctx.enter_context(
                tc.tile_pool(name="ps_flex", bufs=2, space="PSUM")
            )
            ps_s = ctx.enter_context(tc.tile_pool(name="ps_s", bufs=2, space="PSUM"))
            ps_av = ctx.enter_context(tc.tile_pool(name="ps_av", bufs=1, space="PSUM"))

            dma = nc.sync
            dma2 = nc.gpsimd  # second DMA-issue queue for the normalize path

            # constants
            ones_row = consts.tile([1, LP], bf16)
            nc.vector.memset(ones_row[:, 0:L], 1.0)
            nc.vector.memset(ones_row[:, L:LP], 0.0)
            bq_all = consts.tile([128, 12], f32, name="bq_all")
            dma.dma_start(out=bq_all, in_=bqkP[:, :])
            cs_all = consts.tile([65, 24], f32, name="cs_all")
            dma.dma_start(out=cs_all, in_=csP[:, :])
            bo_sb = consts.tile([128, 6], f32, name="bo_sb")
            dma.dma_start(out=bo_sb, in_=boP[:, :])

            ot_pairs = {}
            for b in range(BPC):
                for pair in range(6):
                    ot_pairs[(b, pair)] = ot_pool.tile(
                        [128, L], bf16, tag="ot", name="ot_pair"
                    )

            # ---- persistent x in SBUF: one [128, 6, LP] tile per batch ---
            xa_t = {}
            for b in range(BPC):
                t = xa_pool.tile([128, 6, LP], bf16, tag="xa", name="xa_tile")
                dma.dma_start(out=t, in_=xaP[b])
                xa_t[b] = t

            # output-projection weights, prefetched so the O phase starts
            # without a DMA stall
            wo_sb = wo_pool.tile([128, 6, F], bf16, name="wo_sb")
            dma.dma_start(out=wo_sb, in_=woP[:, :, :])

            # q sub-tiles for projections (moving dim <= 512); only token
            # 1024 of the padded tail is real
            qsubs = [(0, 512), (512, 512), (1024, 1)]
            # attention q tiling: main [0,1024) in 2 psum-bank halves + tail col
            def st_slices():
                return [(0, 512), (512, 512)]

            # ---- v projections, per 3-pair group ------------------------
            # wv columns are grouped per head: h*65 + (0..63 -> wv, 64 -> ones)
            vaug = {}      # (b, g) -> [128, NKB, 390]

            def emit_vproj(g):
                wv_sb = wv_pool.tile([128, 7, 390], bf16, tag="wv")
                dma.dma_start(out=wv_sb, in_=wvP[g])
                for b in range(BPC):
                    va = vaug_pool.tile([128, NKB, 390], bf16, tag="vaug")
                    for tb in range(NKB):
                        ps = ps_flex.tile([128, 512], f32, tag="flex", name="ps_v")
                        for c in range(6):
                            nc.tensor.matmul(
                                ps[:, 0:390],
                                xa_t[b][:, c, tb * 128 : (tb + 1) * 128],
                                wv_sb[:, c, :],
                                start=(c == 0),
                                stop=False,
                            )
                        nc.tensor.matmul(
                            ps[:, 0:390],
                            ones_row[:, tb * 128 : (tb + 1) * 128],
                            wv_sb[0:1, 6, :],
                            start=False,
                            stop=True,
                        )
                        nc.scalar.activation(va[:, tb, :], ps[:, 0:390], AF.Copy)
                    vaug[(b, g)] = va

            # ---- main loop over head pairs ------------------------------
            for pair in range(6):
                g = pair // 3
                if pair % 3 == 0:
                    emit_vproj(g)

                # qT/kT projections for this pair, both batches
                wq_sb = wqk_pool.tile([128, 6, 128], bf16, tag="wq")
                wk_sb = wqk_pool.tile([128, 6, 128], bf16, tag="wk")
                dma.dma_start(out=wq_sb, in_=wqP[pair])
                dma.dma_start(out=wk_sb, in_=wkP[pair])

                qT = {}
                kT = {}
                for b in range(BPC):
                    qt = qkt_pool.tile([128, LP], bf16, tag="qT")
                    kt = qkt_pool.tile([128, LP], bf16, tag="kT")
                    # k-pad columns are read by the j=8 S matmul (masked to
                    # zero afterwards) - keep them finite
                    nc.vector.memset(kt[:, L:LP], 0.0)
                    for (dst, w_sb, scl, bi) in (
                        (qt, wq_sb, 0.125, 0),
                        (kt, wk_sb, 1.0, 1),
                    ):
                        for (q0, qw) in qsubs:
                            psq = ps_flex.tile(
                                [128, 512], f32, tag="flex", name="ps_qk"
                            )
                            for c in range(6):
                                nc.tensor.matmul(
                                    psq[:, 0:qw],
                                    w_sb[:, c, :],
                                    xa_t[b][:, c, q0 : q0 + qw],
                                    start=(c == 0), stop=(c == 5),
                                )
                            # relu(scale*xw + scale*b); the reference's +eps
                            # here is dropped (~1e-7 relative effect)
                            nc.scalar.activation(
                                dst[:, q0 : q0 + qw], psq[:, 0:qw], AF.Relu,
                                scale=scl,
                                bias=bq_all[:, 2 * pair + bi : 2 * pair + bi + 1],
                            )
                    qT[b] = qt
                    kT[b] = kt

                av_sbs = {}
                for hh in range(2):
                    h = pair * 2 + hh
                    r0 = hh * 64
                    # mask tile for this head (shared across batches)
                    mk = mask_pool.tile(
                        [128, NKB, QM], bf16, tag="mask", name="mask_tile"
                    )
                    dma.dma_start(out=mk, in_=maskP[h])
                    mkt = mtail_pool.tile([128, NKB], bf16, tag="mtail")
                    dma.dma_start(out=mkt, in_=mask_tail[h])

                    for b in range(BPC):
                        va = vaug[(b, pair // 3)]
                        vc0 = (pair % 3) * 130 + hh * 65

                        av = ps_av.tile([65, QM], f32, tag="ps_av")
                        ptl = ps_flex.tile(
                            [128, 512], f32, tag="flex", name="ps_tails"
                        )
                        stail = ptl[:, 0:NKB]
                        avt = ptl[0:65, NKB : NKB + 1]
                        mtt = mttail_pool.tile([128, NKB], bf16, tag="mttail")

                        # software-pipelined: AV_j issues after S_{j+1} so
                        # the PE never waits on the DVE mask-multiply
                        def emit_s(j):
                            lhs_k = kT[b][r0 : r0 + 64, j * 128 : (j + 1) * 128]
                            st = ps_s.tile([128, QM], f32, tag="ps_s")
                            for (q0, qw) in st_slices():
                                nc.tensor.matmul(
                                    st[:, q0 : q0 + qw],
                                    lhs_k,
                                    qT[b][r0 : r0 + 64, q0 : q0 + qw],
                                    start=True, stop=True,
                                )
                            # tail column q=1024 (shares the kT weights)
                            nc.tensor.matmul(
                                stail[:, j : j + 1],
                                lhs_k,
                                qT[b][r0 : r0 + 64, QM : QM + 1],
                                start=True, stop=True,
                            )
                            # masked scores -> bf16
                            mt = mt_pool.tile([128, QM], bf16, tag="mt")
                            nc.vector.tensor_mul(mt, st, mk[:, j, :])
                            return mt

                        def emit_av(j, mt):
                            # AV accumulation (row 64 = rowsum via ones col)
                            for (q0, qw) in st_slices():
                                nc.tensor.matmul(
                                    av[:, q0 : q0 + qw],
                                    va[:, j, vc0 : vc0 + 65],
                                    mt[:, q0 : q0 + qw],
                                    start=(j == 0), stop=(j == NKB - 1),
                                )

                        mt_prev = emit_s(0)
                        for j in range(1, NKB):
                            mt_j = emit_s(j)
                            emit_av(j - 1, mt_prev)
                            mt_prev = mt_j
                        emit_av(NKB - 1, mt_prev)

                        # tail: masked scores + AV
                        nc.vector.tensor_mul(mtt, stail, mkt)
                        for j in range(NKB):
                            nc.tensor.matmul(
                                avt,
                                va[:, j, vc0 : vc0 + 65],
                                mtt[:, j : j + 1],
                                start=(j == 0), stop=(j == NKB - 1),
                            )

                        # drain AV psum to SBUF (frees the banks for the
                        # next head while the normalize chain runs)
                        av_sb = avsb_pool.tile([65, L], f32, tag="avsb")
                        nc.scalar.activation(av_sb[:, 0:QM], av, AF.Copy)
                        nc.scalar.activation(av_sb[:, QM : QM + 1], avt, AF.Copy)
                        av_sbs[(hh, b)] = av_sb

                # ---- batched normalization for the pair's 4 (hh, b) -----
                # gather rowsum rows into one tile, one batched reciprocal
                def normalize(combos, slot0):
                    n = len(combos)
                    rs = rs_pool.tile([4, L], f32, tag="rs")
                    for idx, (hh, b) in enumerate(combos):
                        dma2.dma_start(
                            out=rs[idx : idx + 1, :],
                            in_=av_sbs[(hh, b)][64:65, :],
                        )
                    nc.vector.tensor_scalar_add(
                        rs[0:n], rs[0:n], float(L) * EPS
                    )
                    rr = rs_pool.tile([4, L], f32, tag="rr")
                    nc.vector.reciprocal(rr[0:n], rs[0:n])
                    dma2.dma_start(out=rr_dram[slot0 : slot0 + n, :], in_=rr[0:n])
                    for idx, (hh, b) in enumerate(combos):
                        rr_slot = rr_dram[slot0 + idx]
                        rr_bcast_src = bass.AP(
                            tensor=rr_slot.tensor,
                            offset=rr_slot.offset,
                            ap=[[0, 64]] + list(rr_slot.ap),
                        )
                        rrb = rrb_pool.tile([64, L], f32, tag="rrb")
                        dma2.dma_start(out=rrb, in_=rr_bcast_src)
                        hg = (pair % 3) * 2 + hh
                        ci = b * 12 + g * 6 + hg
                        r0h = hh * 64
                        nc.vector.scalar_tensor_tensor(
                            ot_pairs[(b, pair)][r0h : r0h + 64, :],
                            av_sbs[(hh, b)][0:64, :],
                            cs_all[0:64, ci : ci + 1],
                            rrb,
                            op0=mybir.AluOpType.add,
                            op1=mybir.AluOpType.mult,
                        )

                if pair < 5:
                    normalize(
                        [(hh, b) for hh in range(2) for b in range(BPC)],
                        (pair % 2) * 4,
                    )
                else:
                    # last pair: per-batch so the O projection of b=0 isn't
                    # gated on b=1's normalize chain
                    normalize([(0, 0), (1, 0)], 4)
                    normalize([(0, 1), (1, 1)], 6)

            # ---- output projection: yT = wo^T @ O^T + bo ----------------
            ctx.close()
            ctx = octx.enter_context(ExitStack())
            y_pool = ctx.enter_context(tc.tile_pool(name="y", bufs=2))
            ps_y = ctx.enter_context(tc.tile_pool(name="ps_y", bufs=2, space="PSUM"))

            oq_tiles = [(0, 512), (512, 512), (1024, 1)]
            for b in range(BPC):
                y_tile = y_pool.tile([128, 6, L], bf16, tag="y", name="y_tile")
                for (q0, qw) in oq_tiles:
                    for fc in range(6):
                        psy = ps_y.tile([128, 512], f32, tag="ps_y")
                        for hc in range(6):
                            nc.tensor.matmul(
                                psy[:, 0:qw],
                                wo_sb[:, hc, fc * 128 : (fc + 1) * 128],
                                ot_pairs[(b, hc)][:, q0 : q0 + qw],
                                start=(hc == 0), stop=(hc == 5),
                            )
                        # drain with bo fused as the per-partition bias
                        nc.scalar.activation(
                            y_tile[:, fc, q0 : q0 + qw], psy[:, 0:qw],
                            AF.Identity, bias=bo_sb[:, fc : fc + 1],
                        )
                dma.dma_start(out=yT[b], in_=y_tile)

    _split_matmul_waits(nc)
    return nc


def _split_matmul_waits(nc):
    """Walrus TPB instruction structs encode a limited number of sync waits
    (the fp32 LDWEIGHTS+MATMUL pair can take none beyond its update).  Hoist
    excess waits onto same-engine NoOps inserted just before each
    instruction."""
    import bass_rust
    from concourse import mybir

    n = 0
    for f in nc.m.functions:
        for blk in f.blocks:
            insts = blk.instructions
            out = []
            for inst in insts:
                si = inst.sync_info
                tname = type(inst).__name__
                if si is not None and len(si.on_wait) > 0 and tname != "InstISA":
                    cap = 0 if tname == "InstMatmult" else 1
                    waits = list(si.on_wait)
                    if len(waits) > cap:
                        hoist = waits[: len(waits) - cap]
                        keep = waits[len(waits) - cap :]
                        for w in hoist:
                            nop = mybir.InstNoOp(
                                name=f"I-mmw-{n}", ins=[], outs=[]
                            )
                            n += 1
                            nop.engine = inst.engine
                            nop.sync_info = bass_rust.SyncInfo(
                                on_wait=[w], on_update=[]
                            )
                            out.append(nop)
                        inst.sync_info = bass_rust.SyncInfo(
                            on_wait=keep, on_update=list(si.on_update)
                        )
                out.append(inst)
            insts[:] = out
    return n


def _dist_index():
    gi = np.arange(NB)
    gj = np.arange(NB)
    idx = (
        (gi[:, None, None, None] - gi[None, None, :, None] + NB) * 2 * NB
        + gj[None, :, None, None]
        - gj[None, None, None, :]
        + NB
    )
    return idx.reshape(-1).astype(np.int32)


def _host_prep(x, wq, bq, wk, bk, wv, bv, wo, bo, toeplitz_params):
    import ml_dtypes

    f4 = np.float32
    bf = ml_dtypes.bfloat16
    x = np.asarray(x, f4)
    L0 = NB * NB

    # x, transposed to [F, L], padded to LP, packed [128, 6, LP]
    xs = np.transpose(x, (0, 2, 1))  # [B, F, L]
    xaP = np.zeros((B, 128, 6, LP), bf)
    xaP[:, :, :, :L] = xs.reshape(B, 6, 128, L).transpose(0, 2, 1, 3).astype(bf)

    wq_flat = np.asarray(wq, f4).reshape(F, F)
    wk_flat = np.asarray(wk, f4).reshape(F, F)
    wqP = np.ascontiguousarray(
        wq_flat.reshape(6, 128, 6, 128).transpose(2, 1, 0, 3).astype(bf)
    )
    wkP = np.ascontiguousarray(
        wk_flat.reshape(6, 128, 6, 128).transpose(2, 1, 0, 3).astype(bf)
    )

    wvr = np.asarray(wv, f4)
    bvr = np.asarray(bv, f4)
    wv_aug = np.zeros((FA, H * 65), f4)
    for h in range(H):
        wv_aug[:F, h * 65 : h * 65 + 64] = wvr[:, h, :]
        wv_aug[F, h * 65 : h * 65 + 64] = bvr[h]
        wv_aug[F, h * 65 + 64] = 1.0
    wvP = np.zeros((2, 128, 7, 390), bf)
    wvP[:, :, :6, :] = (
        wv_aug[:F].reshape(6, 128, 2, 390).transpose(2, 1, 0, 3).astype(bf)
    )
    wvP[:, 0, 6, :] = wv_aug[F].reshape(2, 390).astype(bf)

    wo_flat = np.asarray(wo, f4).reshape(H * D, F)
    woP = np.ascontiguousarray(
        wo_flat.reshape(6, 128, F).transpose(1, 0, 2).astype(bf)
    )
    boP = np.ascontiguousarray(np.asarray(bo, f4).reshape(6, 128).T)

    bqs = (np.asarray(bq, f4).reshape(F) * 0.125).reshape(6, 128)
    bks = np.asarray(bk, f4).reshape(F).reshape(6, 128)
    bqkP = np.zeros((128, 12), f4)
    bqkP[:, 0::2] = bqs.T
    bqkP[:, 1::2] = bks.T

    # gathered |toeplitz| mask, padded (CLS row/col of ones), transposed,
    # k padded to 1152 with zeros, packed [H, 128, NKB, QM]
    tp = np.asarray(toeplitz_params, f4)
    tm = np.abs(tp[:, _dist_index()]).reshape(H, L0, L0)
    tm_full = np.ones((H, L, L), f4)
    tm_full[:, 1:, 1:] = tm
    maskT = np.zeros((H, LP, L), bf)
    maskT[:, :L, :] = np.transpose(tm_full, (0, 2, 1)).astype(bf)
    maskP = np.ascontiguousarray(
        maskT[:, :, :QM].reshape(H, NKB, 128, QM).transpose(0, 2, 1, 3)
    )
    maskT_tail = np.ascontiguousarray(
        maskT[:, :, QM].reshape(H, NKB, 128).transpose(0, 2, 1)
    )

    xsum = x.sum(axis=1)  # [B, F]
    cs = np.einsum("bf,fhd->bhd", xsum, wvr) + L * bvr[None]  # [B, H, 64]
    cs_full = np.concatenate(
        [cs, np.full((B, H, 1), float(L), np.float32)], axis=2
    ) * np.float32(EPS)  # [B, H, 65]

    shared = dict(
        bqkP=bqkP,
        wqP=wqP,
        wkP=wkP,
        wvP=wvP,
        woP=woP,
        boP=boP,
        maskP=maskP,
        maskT_tail=maskT_tail,
    )
    in_maps = []
    for c in range(NCORES):
        m = dict(shared)
        m["xaP"] = np.ascontiguousarray(xaP[c * BPC : (c + 1) * BPC])
        csP = np.zeros((65, 24), f4)
        for b in range(BPC):
            for g in range(2):
                for hg in range(6):
                    csP[:, b * 12 + g * 6 + hg] = cs_full[
                        c * BPC + b, 6 * g + hg, :
                    ]
        m["csP"] = csP
        in_maps.append(m)
    return in_maps


def _get_program():
    global _PROG
    if _PROG is None:
        _PROG = _build_program()
    return _PROG


def run(trace=False, **inputs):
    from concourse.bass_utils import run_bass_kernel_spmd

    nc = _get_program()
    in_maps = _host_prep(**inputs)
    res = run_bass_kernel_spmd(nc, in_maps, list(range(NCORES)), trace=trace)
    outs = []
    for c in range(NCORES):
        yt = np.asarray(res.results[c]["yT"], dtype=np.float32)  # [BPC,128,6,L]
        # y[b, l, fc*128 + p] = yt[b, p, fc, l]
        outs.append(yt.transpose(0, 3, 2, 1).reshape(BPC, L, F))
    y = np.concatenate(outs, axis=0).astype(np.float32)
    return y, res


def kernel(**inputs):
    y, _ = run(trace=False, **inputs)
    return y


# revision 35
# speedup vs baseline: 1.1485x; 1.0040x over previous
"""Trainium2 Bass kernel for relu-kernelized multi-head attention with a
per-head Toeplitz relative-position mask (sparse_attention problem).

Contract: kernel(**inputs) takes FULL unsharded inputs (numpy), returns the
FULL output [16, 1025, 768]. Internally: data-parallel over batch across 8
NeuronCores (2 batches/core), identical SPMD program, per-core inputs differ
only in the x shard.

Math (per batch b):
  q = relu((x@wq + bq)/8) + eps ; k = relu(x@wk + bk) + eps ; v = x@wv + bv
  S[q,k] = sum_d q*k ;  attn = S*|tm| + eps ; attn /= rowsum ; out = attn@v
  y = out@wo + bo

Perf structure (v6):
  - all matmul operands bf16 (PE 1 cycle/row vs fp32's 4), fp32 PSUM.
  - every logical load is ONE DMA: host pre-packs all tensors in the exact
    [partition, ...] SBUF layout (DMA issue on the sync queue costs ~650ns
    each - the v2 kernel spent >160us there).
  - S/AV j-loop is software-pipelined (AV_j after S_{j+1}) so the PE never
    waits on the DVE mask-multiply.
  - row-normalization batched per head pair: one [4,L] reciprocal, DMA
    partition-broadcast of 1/r via a DRAM bounce, all on the gpsimd queue.
  - attention outputs stay in SBUF as 12 [128,L] bf16 head-pair tiles
    consumed directly by the O projection; output shipped bf16.
  - the q/k "+eps" of the reference is dropped (~1e-7 relative effect); the
    attention-level eps is kept via the cs rank-1 correction and the
    rowsum + L*eps denominator.
"""

import os
import sys

sys.path.insert(0, "/opt/trn_rl_repo")

import numpy as np

B, L, F, H, D = 16, 1025, 768, 12, 64
NB = 32
EPS = 1e-8
LP = 1152           # padded token count (9 * 128)
NKB = 9             # k blocks of 128
QM = 1024           # main q width (q tail = 1 col, index 1024)
FA = F + 1          # augmented contraction (ones row)
NCORES = 8
BPC = B // NCORES   # batches per core

_PROG = None


def _build_program():
    import concourse.bass as bass
    import concourse.tile as tile
    from concourse import mybir

    f32 = mybir.dt.float32
    bf16 = mybir.dt.bfloat16
    AF = mybir.ActivationFunctionType

    nc = bass.Bass()

    xaP = nc.declare_dram_parameter("xaP", [BPC, 128, 6, LP], bf16, isOutput=False)
    wqP = nc.declare_dram_parameter("wqP", [6, 128, 6, 128], bf16, isOutput=False)
    wkP = nc.declare_dram_parameter("wkP", [6, 128, 6, 128], bf16, isOutput=False)
    wvP = nc.declare_dram_parameter("wvP", [2, 128, 7, 390], bf16, isOutput=False)
    woP = nc.declare_dram_parameter("woP", [128, 6, F], bf16, isOutput=False)
    boP = nc.declare_dram_parameter("boP", [128, 6], f32, isOutput=False)
    bqkP = nc.declare_dram_parameter("bqkP", [128, 12], f32, isOutput=False)
    csP = nc.declare_dram_parameter("csP", [65, 24], f32, isOutput=False)
    maskP = nc.declare_dram_parameter(
        "maskP", [H, 128, NKB, QM], bf16, isOutput=False
    )
    mask_tail = nc.declare_dram_parameter(
        "maskT_tail", [H, 128, NKB], bf16, isOutput=False
    )
    yT = nc.declare_dram_parameter("yT", [BPC, 128, 6, L], bf16, isOutput=True)

    rr_dram = nc.dram_tensor("rr_dram", [8, L], f32)

    with tile.TileContext(nc) as tc:
        from contextlib import ExitStack

        with ExitStack() as octx:
            consts = octx.enter_context(tc.tile_pool(name="consts", bufs=1))
            # attention outputs, SBUF-resident across phases: 12 tiles
            # [128, L] bf16, one per (batch, head-pair); rows 0:64 = even
            # head, 64:128 = odd head of the pair
            ot_pool = octx.enter_context(tc.tile_pool(name="ot", bufs=2 * 6))
            wo_pool = octx.enter_context(tc.tile_pool(name="wo", bufs=1))
            ctx = octx.enter_context(ExitStack())
            xa_pool = ctx.enter_context(tc.tile_pool(name="xa", bufs=2))
            wqk_pool = ctx.enter_context(tc.tile_pool(name="wqk", bufs=2))
            wv_pool = ctx.enter_context(tc.tile_pool(name="wv", bufs=2))
            qkt_pool = ctx.enter_context(tc.tile_pool(name="qkt", bufs=2))
            vaug_pool = ctx.enter_context(tc.tile_pool(name="vaug", bufs=4))
            mask_pool = ctx.enter_context(tc.tile_pool(name="mask", bufs=2))
            mtail_pool = ctx.enter_context(tc.tile_pool(name="mtail", bufs=2))
            mt_pool = ctx.enter_context(tc.tile_pool(name="mt", bufs=4))
            std_pool = ctx.enter_context(tc.tile_pool(name="std", bufs=2))
            mttail_pool = ctx.enter_context(tc.tile_pool(name="mttail", bufs=2))
            rs_pool = ctx.enter_context(tc.tile_pool(name="rs", bufs=1))
            rrb_pool = ctx.enter_context(tc.tile_pool(name="rrb", bufs=3))
            avsb_pool = ctx.enter_context(tc.tile_pool(name="avsb", bufs=4))

            # flex pool: [128,512] tiles time-shared between projection psums
            # (2-deep so the activation drain doesn't stall the next matmul
            # group) and the per-head tail psum (stail+avt live in a slice)
            ps_flex = ctx.enter_context(
                tc.tile_pool(name="ps_flex", bufs=2, space="PSUM")
            )
            ps_s = ctx.enter_context(tc.tile_pool(name="ps_s", bufs=2, space="PSUM"))
            ps_av = ctx.enter_context(tc.tile_pool(name="ps_av", bufs=1, space="PSUM"))

            # three DMA-issue queues: sync carries x/weights/outputs, the
            # scalar queue carries the big mask transfers, gpsimd carries
            # the normalize path + upfront small loads
            dma = nc.sync
            dma2 = nc.gpsimd
            dma3 = nc.scalar

            # constants
            ones_row = consts.tile([1, LP], bf16)
            nc.vector.memset(ones_row[:, 0:L], 1.0)
            nc.vector.memset(ones_row[:, L:LP], 0.0)
            bq_all = consts.tile([128, 12], f32, name="bq_all")
            dma2.dma_start(out=bq_all, in_=bqkP[:, :])
            cs_all = consts.tile([65, 24], f32, name="cs_all")
            dma2.dma_start(out=cs_all, in_=csP[:, :])
            bo_sb = consts.tile([128, 6], f32, name="bo_sb")
            dma2.dma_start(out=bo_sb, in_=boP[:, :])

            ot_pairs = {}
            for b in range(BPC):
                for pair in range(6):
                    ot_pairs[(b, pair)] = ot_pool.tile(
                        [128, L], bf16, tag="ot", name="ot_pair"
                    )

            # ---- persistent x in SBUF: one [128, 6, LP] tile per batch ---
            xa_t = {}
            for b in range(BPC):
                t = xa_pool.tile([128, 6, LP], bf16, tag="xa", name="xa_tile")
                dma.dma_start(out=t, in_=xaP[b])
                xa_t[b] = t

            # output-projection weights, prefetched so the O phase starts
            # without a DMA stall
            wo_sb = wo_pool.tile([128, 6, F], bf16, name="wo_sb")
            dma2.dma_start(out=wo_sb, in_=woP[:, :, :])

            # q sub-tiles for projections (moving dim <= 512); only token
            # 1024 of the padded tail is real
            qsubs = [(0, 512), (512, 512), (1024, 1)]
            # attention q tiling: main [0,1024) in 2 psum-bank halves + tail col
            def st_slices():
                return [(0, 512), (512, 512)]

            # ---- v projections, per 3-pair group ------------------------
            # wv columns are grouped per head: h*65 + (0..63 -> wv, 64 -> ones)
            vaug = {}      # (b, g) -> [128, NKB, 390]

            def emit_vproj(g):
                wv_sb = wv_pool.tile([128, 7, 390], bf16, tag="wv")
                dma.dma_start(out=wv_sb, in_=wvP[g])
                for b in range(BPC):
                    va = vaug_pool.tile([128, NKB, 390], bf16, tag="vaug")
                    for tb in range(NKB):
                        ps = ps_flex.tile([128, 512], f32, tag="flex", name="ps_v")
                        for c in range(6):
                            nc.tensor.matmul(
                                ps[:, 0:390],
                                xa_t[b][:, c, tb * 128 : (tb + 1) * 128],
                                wv_sb[:, c, :],
                                start=(c == 0),
                                stop=False,
                            )
                        nc.tensor.matmul(
                            ps[:, 0:390],
                            ones_row[:, tb * 128 : (tb + 1) * 128],
                            wv_sb[0:1, 6, :],
                            start=False,
                            stop=True,
                        )
                        nc.scalar.activation(va[:, tb, :], ps[:, 0:390], AF.Copy)
                    vaug[(b, g)] = va

            # ---- main loop over head pairs ------------------------------
            for pair in range(6):
                g = pair // 3
                if pair % 3 == 0:
                    emit_vproj(g)

                # qT/kT projections for this pair, both batches
                wq_sb = wqk_pool.tile([128, 6, 128], bf16, tag="wq")
                wk_sb = wqk_pool.tile([128, 6, 128], bf16, tag="wk")
                dma.dma_start(out=wq_sb, in_=wqP[pair])
                dma.dma_start(out=wk_sb, in_=wkP[pair])

                qT = {}
                kT = {}
                for b in range(BPC):
                    qt = qkt_pool.tile([128, LP], bf16, tag="qT")
                    kt = qkt_pool.tile([128, LP], bf16, tag="kT")
                    # k-pad columns are read by the j=8 S matmul (masked to
                    # zero afterwards) - keep them finite
                    nc.vector.memset(kt[:, L:LP], 0.0)
                    for (dst, w_sb, scl, bi) in (
                        (qt, wq_sb, 0.125, 0),
                        (kt, wk_sb, 1.0, 1),
                    ):
                        for (q0, qw) in qsubs:
                            psq = ps_flex.tile(
                                [128, 512], f32, tag="flex", name="ps_qk"
                            )
                            for c in range(6):
                                nc.tensor.matmul(
                                    psq[:, 0:qw],
                                    w_sb[:, c, :],
                                    xa_t[b][:, c, q0 : q0 + qw],
                                    start=(c == 0), stop=(c == 5),
                                )
                            # relu(scale*xw + scale*b); the reference's +eps
                            # here is dropped (~1e-7 relative effect)
                            nc.scalar.activation(
                                dst[:, q0 : q0 + qw], psq[:, 0:qw], AF.Relu,
                                scale=scl,
                                bias=bq_all[:, 2 * pair + bi : 2 * pair + bi + 1],
                            )
                    qT[b] = qt
                    kT[b] = kt

                av_sbs = {}
                for hh in range(2):
                    h = pair * 2 + hh
                    r0 = hh * 64
                    # mask tile for this head (shared across batches)
                    mk = mask_pool.tile(
                        [128, NKB, QM], bf16, tag="mask", name="mask_tile"
                    )
                    dma3.dma_start(out=mk, in_=maskP[h])
                    mkt = mtail_pool.tile([128, NKB], bf16, tag="mtail")
                    dma3.dma_start(out=mkt, in_=mask_tail[h])

                    for b in range(BPC):
                        va = vaug[(b, pair // 3)]
                        vc0 = (pair % 3) * 130 + hh * 65

                        av = ps_av.tile([65, QM], f32, tag="ps_av")
                        ptl = ps_flex.tile(
                            [128, 512], f32, tag="flex", name="ps_tails"
                        )
                        stail = ptl[:, 0:NKB]
                        avt = ptl[0:65, NKB : NKB + 1]
                        mtt = mttail_pool.tile([128, NKB], bf16, tag="mttail")

                        # software-pipelined at depth 2: AV_j issues after
                        # S_{j+2}, so the PE never waits on the scalar
                        # drain + DVE 2x-mode mask-multiply chain
                        def emit_s(j):
                            lhs_k = kT[b][r0 : r0 + 64, j * 128 : (j + 1) * 128]
                            st = ps_s.tile([128, QM], f32, tag="ps_s")
                            for (q0, qw) in st_slices():
                                nc.tensor.matmul(
                                    st[:, q0 : q0 + qw],
                                    lhs_k,
                                    qT[b][r0 : r0 + 64, q0 : q0 + qw],
                                    start=True, stop=True,
                                )
                            # tail column q=1024 (shares the kT weights)
                            nc.tensor.matmul(
                                stail[:, j : j + 1],
                                lhs_k,
                                qT[b][r0 : r0 + 64, QM : QM + 1],
                                start=True, stop=True,
                            )
                            # drain scores to bf16 on the scalar engine so
                            # the DVE multiply runs in 2x mode (all-16-bit)
                            std = std_pool.tile([128, QM], bf16, tag="std")
                            nc.scalar.activation(std, st, AF.Copy)
                            mt = mt_pool.tile([128, QM], bf16, tag="mt")
                            nc.vector.tensor_mul(mt, std, mk[:, j, :])
                            return mt

                        def emit_av(j, mt):
                            # AV accumulation (row 64 = rowsum via ones col)
                            for (q0, qw) in st_slices():
                                nc.tensor.matmul(
                                    av[:, q0 : q0 + qw],
                                    va[:, j, vc0 : vc0 + 65],
                                    mt[:, q0 : q0 + qw],
                                    start=(j == 0), stop=(j == NKB - 1),
                                )

                        mts = [emit_s(0), emit_s(1)]
                        for j in range(2, NKB):
                            mts.append(emit_s(j))
                            emit_av(j - 2, mts[j - 2])
                        emit_av(NKB - 2, mts[NKB - 2])
                        emit_av(NKB - 1, mts[NKB - 1])

                        # tail: masked scores + AV
                        nc.vector.tensor_mul(mtt, stail, mkt)
                        for j in range(NKB):
                            nc.tensor.matmul(
                                avt,
                                va[:, j, vc0 : vc0 + 65],
                                mtt[:, j : j + 1],
                                start=(j == 0), stop=(j == NKB - 1),
                            )

                        # drain AV psum to SBUF (frees the banks for the
                        # next head while the normalize chain runs)
                        av_sb = avsb_pool.tile([65, L], f32, tag="avsb")
                        nc.scalar.activation(av_sb[:, 0:QM], av, AF.Copy)
                        nc.scalar.activation(av_sb[:, QM : QM + 1], avt, AF.Copy)
                        av_sbs[(hh, b)] = av_sb

                # ---- batched normalization for the pair's 4 (hh, b) -----
                # gather rowsum rows into one tile, one batched reciprocal
                def normalize(combos, slot0):
                    n = len(combos)
                    rs = rs_pool.tile([4, L], f32, tag="rs")
                    for idx, (hh, b) in enumerate(combos):
                        dma2.dma_start(
                            out=rs[idx : idx + 1, :],
                            in_=av_sbs[(hh, b)][64:65, :],
                        )
                    nc.vector.tensor_scalar_add(
                        rs[0:n], rs[0:n], float(L) * EPS
                    )
                    rr = rs_pool.tile([4, L], f32, tag="rr")
                    nc.vector.reciprocal(rr[0:n], rs[0:n])
                    dma2.dma_start(out=rr_dram[slot0 : slot0 + n, :], in_=rr[0:n])
                    for idx, (hh, b) in enumerate(combos):
                        rr_slot = rr_dram[slot0 + idx]
                        rr_bcast_src = bass.AP(
                            tensor=rr_slot.tensor,
                            offset=rr_slot.offset,
                            ap=[[0, 64]] + list(rr_slot.ap),
                        )
                        rrb = rrb_pool.tile([64, L], f32, tag="rrb")
                        dma2.dma_start(out=rrb, in_=rr_bcast_src)
                        hg = (pair % 3) * 2 + hh
                        ci = b * 12 + g * 6 + hg
                        r0h = hh * 64
                        nc.vector.scalar_tensor_tensor(
                            ot_pairs[(b, pair)][r0h : r0h + 64, :],
                            av_sbs[(hh, b)][0:64, :],
                            cs_all[0:64, ci : ci + 1],
                            rrb,
                            op0=mybir.AluOpType.add,
                            op1=mybir.AluOpType.mult,
                        )

                if pair < 5:
                    normalize(
                        [(hh, b) for hh in range(2) for b in range(BPC)],
                        (pair % 2) * 4,
                    )
                else:
                    # last pair: per-batch so the O projection of b=0 isn't
                    # gated on b=1's normalize chain
                    normalize([(0, 0), (1, 0)], 4)
                    normalize([(0, 1), (1, 1)], 6)

            # ---- output projection: yT = wo^T @ O^T + bo ----------------
            ctx.close()
            ctx = octx.enter_context(ExitStack())
            y_pool = ctx.enter_context(tc.tile_pool(name="y", bufs=3))
            ps_y = ctx.enter_context(tc.tile_pool(name="ps_y", bufs=2, space="PSUM"))

            oq_tiles = [(0, 512), (512, 512), (1024, 1)]
            for b in range(BPC):
                for fc in range(6):
                    y_tile = y_pool.tile([128, L], bf16, tag="y", name="y_tile")
                    for (q0, qw) in oq_tiles:
                        psy = ps_y.tile([128, 512], f32, tag="ps_y")
                        for hc in range(6):
                            nc.tensor.matmul(
                                psy[:, 0:qw],
                                wo_sb[:, hc, fc * 128 : (fc + 1) * 128],
                                ot_pairs[(b, hc)][:, q0 : q0 + qw],
                                start=(hc == 0), stop=(hc == 5),
                            )
                        # drain with bo fused as the per-partition bias
                        nc.scalar.activation(
                            y_tile[:, q0 : q0 + qw], psy[:, 0:qw],
                            AF.Identity, bias=bo_sb[:, fc : fc + 1],
                        )
                    dma.dma_start(out=yT[b][:, fc, :], in_=y_tile)

    _split_matmul_waits(nc)
    return nc


def _split_matmul_waits(nc):
    """Walrus TPB instruction structs encode a limited number of sync waits
    (the fp32 LDWEIGHTS+MATMUL pair can take none beyond its update).  Hoist
    excess waits onto same-engine NoOps inserted just before each
    instruction."""
    import bass_rust
    from concourse import mybir

    n = 0
    for f in nc.m.functions:
        for blk in f.blocks:
            insts = blk.instructions
            out = []
            for inst in insts:
                si = inst.sync_info
                tname = type(inst).__name__
                if si is not None and len(si.on_wait) > 0 and tname != "InstISA":
                    cap = 0 if tname == "InstMatmult" else 1
                    waits = list(si.on_wait)
                    if len(waits) > cap:
                        hoist = waits[: len(waits) - cap]
                        keep = waits[len(waits) - cap :]
                        for w in hoist:
                            nop = mybir.InstNoOp(
                                name=f"I-mmw-{n}", ins=[], outs=[]
                            )
                            n += 1
                            nop.engine = inst.engine
                            nop.sync_info = bass_rust.SyncInfo(
                                on_wait=[w], on_update=[]
                            )
                            out.append(nop)
                        inst.sync_info = bass_rust.SyncInfo(
                            on_wait=keep, on_update=list(si.on_update)
                        )
                out.append(inst)
            insts[:] = out
    return n


def _dist_index():
    gi = np.arange(NB)
    gj = np.arange(NB)
    idx = (
        (gi[:, None, None, None] - gi[None, None, :, None] + NB) * 2 * NB
        + gj[None, :, None, None]
        - gj[None, None, None, :]
        + NB
    )
    return idx.reshape(-1).astype(np.int32)


def _host_prep(x, wq, bq, wk, bk, wv, bv, wo, bo, toeplitz_params):
    import ml_dtypes

    f4 = np.float32
    bf = ml_dtypes.bfloat16
    x = np.asarray(x, f4)
    L0 = NB * NB

    # x, transposed to [F, L], padded to LP, packed [128, 6, LP]
    xs = np.transpose(x, (0, 2, 1))  # [B, F, L]
    xaP = np.zeros((B, 128, 6, LP), bf)
    xaP[:, :, :, :L] = xs.reshape(B, 6, 128, L).transpose(0, 2, 1, 3).astype(bf)

    wq_flat = np.asarray(wq, f4).reshape(F, F)
    wk_flat = np.asarray(wk, f4).reshape(F, F)
    wqP = np.ascontiguousarray(
        wq_flat.reshape(6, 128, 6, 128).transpose(2, 1, 0, 3).astype(bf)
    )
    wkP = np.ascontiguousarray(
        wk_flat.reshape(6, 128, 6, 128).transpose(2, 1, 0, 3).astype(bf)
    )

    wvr = np.asarray(wv, f4)
    bvr = np.asarray(bv, f4)
    wv_aug = np.zeros((FA, H * 65), f4)
    for h in range(H):
        wv_aug[:F, h * 65 : h * 65 + 64] = wvr[:, h, :]
        wv_aug[F, h * 65 : h * 65 + 64] = bvr[h]
        wv_aug[F, h * 65 + 64] = 1.0
    wvP = np.zeros((2, 128, 7, 390), bf)
    wvP[:, :, :6, :] = (
        wv_aug[:F].reshape(6, 128, 2, 390).transpose(2, 1, 0, 3).astype(bf)
    )
    wvP[:, 0, 6, :] = wv_aug[F].reshape(2, 390).astype(bf)

    wo_flat = np.asarray(wo, f4).reshape(H * D, F)
    woP = np.ascontiguousarray(
        wo_flat.reshape(6, 128, F).transpose(1, 0, 2).astype(bf)
    )
    boP = np.ascontiguousarray(np.asarray(bo, f4).reshape(6, 128).T)

    bqs = (np.asarray(bq, f4).reshape(F) * 0.125).reshape(6, 128)
    bks = np.asarray(bk, f4).reshape(F).reshape(6, 128)
    bqkP = np.zeros((128, 12), f4)
    bqkP[:, 0::2] = bqs.T
    bqkP[:, 1::2] = bks.T

    # gathered |toeplitz| mask, padded (CLS row/col of ones), transposed,
    # k padded to 1152 with zeros, packed [H, 128, NKB, QM]
    tp = np.asarray(toeplitz_params, f4)
    tm = np.abs(tp[:, _dist_index()]).reshape(H, L0, L0)
    tm_full = np.ones((H, L, L), f4)
    tm_full[:, 1:, 1:] = tm
    maskT = np.zeros((H, LP, L), bf)
    maskT[:, :L, :] = np.transpose(tm_full, (0, 2, 1)).astype(bf)
    maskP = np.ascontiguousarray(
        maskT[:, :, :QM].reshape(H, NKB, 128, QM).transpose(0, 2, 1, 3)
    )
    maskT_tail = np.ascontiguousarray(
        maskT[:, :, QM].reshape(H, NKB, 128).transpose(0, 2, 1)
    )

    xsum = x.sum(axis=1)  # [B, F]
    cs = np.einsum("bf,fhd->bhd", xsum, wvr) + L * bvr[None]  # [B, H, 64]
    cs_full = np.concatenate(
        [cs, np.full((B, H, 1), float(L), np.float32)], axis=2
    ) * np.float32(EPS)  # [B, H, 65]

    shared = dict(
        bqkP=bqkP,
        wqP=wqP,
        wkP=wkP,
        wvP=wvP,
        woP=woP,
        boP=boP,
        maskP=maskP,
        maskT_tail=maskT_tail,
    )
    in_maps = []
    for c in range(NCORES):
        m = dict(shared)
        m["xaP"] = np.ascontiguousarray(xaP[c * BPC : (c + 1) * BPC])
        csP = np.zeros((65, 24), f4)
        for b in range(BPC):
            for g in range(2):
                for hg in range(6):
                    csP[:, b * 12 + g * 6 + hg] = cs_full[
                        c * BPC + b, 6 * g + hg, :
                    ]
        m["csP"] = csP
        in_maps.append(m)
    return in_maps


def _get_program():
    global _PROG
    if _PROG is None:
        _PROG = _build_program()
    return _PROG


def run(trace=False, **inputs):
    from concourse.bass_utils import run_bass_kernel_spmd

    nc = _get_program()
    in_maps = _host_prep(**inputs)
    res = run_bass_kernel_spmd(nc, in_maps, list(range(NCORES)), trace=trace)
    outs = []
    for c in range(NCORES):
        yt = np.asarray(res.results[c]["yT"], dtype=np.float32)  # [BPC,128,6,L]
        # y[b, l, fc*128 + p] = yt[b, p, fc, l]
        outs.append(yt.transpose(0, 3, 2, 1).reshape(BPC, L, F))
    y = np.concatenate(outs, axis=0).astype(np.float32)
    return y, res


def kernel(**inputs):
    y, _ = run(trace=False, **inputs)
    return y


# revision 38
# speedup vs baseline: 1.2081x; 1.0519x over previous
"""Trainium2 Bass kernel for relu-kernelized multi-head attention with a
per-head Toeplitz relative-position mask (sparse_attention problem).

Contract: kernel(**inputs) takes FULL unsharded inputs (numpy), returns the
FULL output [16, 1025, 768]. Internally: data-parallel over batch across 8
NeuronCores (2 batches/core), identical SPMD program, per-core inputs differ
only in the x shard.

Math (per batch b):
  q = relu((x@wq + bq)/8) + eps ; k = relu(x@wk + bk) + eps ; v = x@wv + bv
  S[q,k] = sum_d q*k ;  attn = S*|tm| + eps ; attn /= rowsum ; out = attn@v
  y = out@wo + bo

Perf structure (v6):
  - all matmul operands bf16 (PE 1 cycle/row vs fp32's 4), fp32 PSUM.
  - every logical load is ONE DMA: host pre-packs all tensors in the exact
    [partition, ...] SBUF layout (DMA issue on the sync queue costs ~650ns
    each - the v2 kernel spent >160us there).
  - S/AV j-loop is software-pipelined (AV_j after S_{j+1}) so the PE never
    waits on the DVE mask-multiply.
  - row-normalization batched per head pair: one [4,L] reciprocal, DMA
    partition-broadcast of 1/r via a DRAM bounce, all on the gpsimd queue.
  - attention outputs stay in SBUF as 12 [128,L] bf16 head-pair tiles
    consumed directly by the O projection; output shipped bf16.
  - the q/k "+eps" of the reference is dropped (~1e-7 relative effect); the
    attention-level eps is kept via the cs rank-1 correction and the
    rowsum + L*eps denominator.
"""

import os
import sys

sys.path.insert(0, "/opt/trn_rl_repo")

import numpy as np

B, L, F, H, D = 16, 1025, 768, 12, 64
NB = 32
EPS = 1e-8
LP = 1152           # padded token count (9 * 128)
NKB = 9             # k blocks of 128
QM = 1024           # main q width (q tail = 1 col, index 1024)
FA = F + 1          # augmented contraction (ones row)
NCORES = 8
BPC = B // NCORES   # batches per core

_PROG = None


def _build_program():
    import concourse.bass as bass
    import concourse.tile as tile
    from concourse import mybir

    f32 = mybir.dt.float32
    bf16 = mybir.dt.bfloat16
    AF = mybir.ActivationFunctionType

    nc = bass.Bass()

    xaP = nc.declare_dram_parameter("xaP", [BPC, 128, 6, LP], bf16, isOutput=False)
    wqP = nc.declare_dram_parameter("wqP", [6, 128, 6, 128], bf16, isOutput=False)
    wkP = nc.declare_dram_parameter("wkP", [6, 128, 6, 128], bf16, isOutput=False)
    wvP = nc.declare_dram_parameter("wvP", [2, 128, 7, 390], bf16, isOutput=False)
    woP = nc.declare_dram_parameter("woP", [128, 6, F], bf16, isOutput=False)
    boP = nc.declare_dram_parameter("boP", [128, 6], f32, isOutput=False)
    bqkP = nc.declare_dram_parameter("bqkP", [128, 12], f32, isOutput=False)
    csP = nc.declare_dram_parameter("csP", [65, 24], f32, isOutput=False)
    maskP = nc.declare_dram_parameter(
        "maskP", [H, 128, NKB, QM], bf16, isOutput=False
    )
    mask_tail = nc.declare_dram_parameter(
        "maskT_tail", [H, 128, NKB], bf16, isOutput=False
    )
    yT = nc.declare_dram_parameter("yT", [BPC, 128, 6, L], bf16, isOutput=True)

    rr_dram = nc.dram_tensor("rr_dram", [8, L], f32)

    with tile.TileContext(nc) as tc:
        from contextlib import ExitStack

        with ExitStack() as octx:
            consts = octx.enter_context(tc.tile_pool(name="consts", bufs=1))
            # attention outputs, SBUF-resident across phases: 12 tiles
            # [128, L] bf16, one per (batch, head-pair); rows 0:64 = even
            # head, 64:128 = odd head of the pair
            ot_pool = octx.enter_context(tc.tile_pool(name="ot", bufs=2 * 6))
            wo_pool = octx.enter_context(tc.tile_pool(name="wo", bufs=1))
            ctx = octx.enter_context(ExitStack())
            xa_pool = ctx.enter_context(tc.tile_pool(name="xa", bufs=2))
            wqk_pool = ctx.enter_context(tc.tile_pool(name="wqk", bufs=2))
            wv_pool = ctx.enter_context(tc.tile_pool(name="wv", bufs=2))
            qkt_pool = ctx.enter_context(tc.tile_pool(name="qkt", bufs=2))
            vaug_pool = ctx.enter_context(tc.tile_pool(name="vaug", bufs=4))
            mask_pool = ctx.enter_context(tc.tile_pool(name="mask", bufs=2))
            mtail_pool = ctx.enter_context(tc.tile_pool(name="mtail", bufs=2))
            mt_pool = ctx.enter_context(tc.tile_pool(name="mt", bufs=4))
            std_pool = ctx.enter_context(tc.tile_pool(name="std", bufs=2))
            mttail_pool = ctx.enter_context(tc.tile_pool(name="mttail", bufs=2))
            rs_pool = ctx.enter_context(tc.tile_pool(name="rs", bufs=1))
            rrb_pool = ctx.enter_context(tc.tile_pool(name="rrb", bufs=3))
            avsb_pool = ctx.enter_context(tc.tile_pool(name="avsb", bufs=4))

            # flex pool: [128,512] tiles time-shared between projection psums
            # (2-deep so the activation drain doesn't stall the next matmul
            # group) and the per-head tail psum (stail+avt live in a slice)
            ps_flex = ctx.enter_context(
                tc.tile_pool(name="ps_flex", bufs=2, space="PSUM")
            )
            ps_s = ctx.enter_context(tc.tile_pool(name="ps_s", bufs=2, space="PSUM"))
            ps_av = ctx.enter_context(tc.tile_pool(name="ps_av", bufs=1, space="PSUM"))

            # three DMA-issue queues: sync carries x/weights/outputs, the
            # scalar queue carries the big mask transfers, gpsimd carries
            # the normalize path + upfront small loads
            dma = nc.sync
            dma2 = nc.gpsimd
            dma3 = nc.scalar

            # constants
            ones_row = consts.tile([1, LP], bf16)
            nc.vector.memset(ones_row[:, 0:L], 1.0)
            nc.vector.memset(ones_row[:, L:LP], 0.0)
            bq_all = consts.tile([128, 12], f32, name="bq_all")
            dma2.dma_start(out=bq_all, in_=bqkP[:, :])
            cs_all = consts.tile([65, 24], f32, name="cs_all")
            dma2.dma_start(out=cs_all, in_=csP[:, :])
            bo_sb = consts.tile([128, 6], f32, name="bo_sb")
            dma2.dma_start(out=bo_sb, in_=boP[:, :])

            ot_pairs = {}
            for b in range(BPC):
                for pair in range(6):
                    ot_pairs[(b, pair)] = ot_pool.tile(
                        [128, L], bf16, tag="ot", name="ot_pair"
                    )

            # ---- persistent x in SBUF: one [128, 6, LP] tile per batch ---
            # per-chunk DMAs so the first V-proj matmul starts as soon as
            # chunk 0 lands instead of waiting for the whole 1.8MB tile
            xa_t = {}
            for b in range(BPC):
                t = xa_pool.tile([128, 6, LP], bf16, tag="xa", name="xa_tile")
                for c in range(6):
                    dma.dma_start(
                        out=t[:, c : c + 1, :], in_=xaP[b, :, c : c + 1, :]
                    )
                xa_t[b] = t

            # output-projection weights, prefetched so the O phase starts
            # without a DMA stall
            wo_sb = wo_pool.tile([128, 6, F], bf16, name="wo_sb")
            dma2.dma_start(out=wo_sb, in_=woP[:, :, :])

            # q sub-tiles for projections (moving dim <= 512); only token
            # 1024 of the padded tail is real
            qsubs = [(0, 512), (512, 512), (1024, 1)]
            # attention q tiling: main [0,1024) in 2 psum-bank halves + tail col
            def st_slices():
                return [(0, 512), (512, 512)]

            # ---- v projections, per 3-pair group ------------------------
            # wv columns are grouped per head: h*65 + (0..63 -> wv, 64 -> ones)
            vaug = {}      # (b, g) -> [128, NKB, 390]

            def emit_vproj(g):
                wv_sb = wv_pool.tile([128, 7, 390], bf16, tag="wv")
                dma3.dma_start(out=wv_sb, in_=wvP[g])
                for b in range(BPC):
                    va = vaug_pool.tile([128, NKB, 390], bf16, tag="vaug")
                    for tb in range(NKB):
                        ps = ps_flex.tile([128, 512], f32, tag="flex", name="ps_v")
                        for c in range(6):
                            nc.tensor.matmul(
                                ps[:, 0:390],
                                xa_t[b][:, c, tb * 128 : (tb + 1) * 128],
                                wv_sb[:, c, :],
                                start=(c == 0),
                                stop=False,
                            )
                        nc.tensor.matmul(
                            ps[:, 0:390],
                            ones_row[:, tb * 128 : (tb + 1) * 128],
                            wv_sb[0:1, 6, :],
                            start=False,
                            stop=True,
                        )
                        nc.scalar.activation(va[:, tb, :], ps[:, 0:390], AF.Copy)
                    vaug[(b, g)] = va

            # ---- main loop over head pairs ------------------------------
            for pair in range(6):
                g = pair // 3
                if pair % 3 == 0:
                    emit_vproj(g)

                # qT/kT projections for this pair, both batches
                wq_sb = wqk_pool.tile([128, 6, 128], bf16, tag="wq")
                wk_sb = wqk_pool.tile([128, 6, 128], bf16, tag="wk")
                dma.dma_start(out=wq_sb, in_=wqP[pair])
                dma.dma_start(out=wk_sb, in_=wkP[pair])

                qT = {}
                kT = {}
                for b in range(BPC):
                    qt = qkt_pool.tile([128, LP], bf16, tag="qT")
                    kt = qkt_pool.tile([128, LP], bf16, tag="kT")
                    # k-pad columns are read by the j=8 S matmul (masked to
                    # zero afterwards) - keep them finite
                    nc.vector.memset(kt[:, L:LP], 0.0)
                    for (dst, w_sb, scl, bi) in (
                        (qt, wq_sb, 0.125, 0),
                        (kt, wk_sb, 1.0, 1),
                    ):
                        for (q0, qw) in qsubs:
                            psq = ps_flex.tile(
                                [128, 512], f32, tag="flex", name="ps_qk"
                            )
                            for c in range(6):
                                nc.tensor.matmul(
                                    psq[:, 0:qw],
                                    w_sb[:, c, :],
                                    xa_t[b][:, c, q0 : q0 + qw],
                                    start=(c == 0), stop=(c == 5),
                                )
                            # relu(scale*xw + scale*b); the reference's +eps
                            # here is dropped (~1e-7 relative effect)
                            nc.scalar.activation(
                                dst[:, q0 : q0 + qw], psq[:, 0:qw], AF.Relu,
                                scale=scl,
                                bias=bq_all[:, 2 * pair + bi : 2 * pair + bi + 1],
                            )
                    qT[b] = qt
                    kT[b] = kt

                av_sbs = {}
                for hh in range(2):
                    h = pair * 2 + hh
                    r0 = hh * 64
                    # mask tile for this head (shared across batches)
                    mk = mask_pool.tile(
                        [128, NKB, QM], bf16, tag="mask", name="mask_tile"
                    )
                    dma3.dma_start(out=mk, in_=maskP[h])
                    mkt = mtail_pool.tile([128, NKB], bf16, tag="mtail")
                    dma3.dma_start(out=mkt, in_=mask_tail[h])

                    for b in range(BPC):
                        va = vaug[(b, pair // 3)]
                        vc0 = (pair % 3) * 130 + hh * 65

                        av = ps_av.tile([65, QM], f32, tag="ps_av")
                        ptl = ps_flex.tile(
                            [128, 512], f32, tag="flex", name="ps_tails"
                        )
                        stail = ptl[:, 0:NKB]
                        avt = ptl[0:65, NKB : NKB + 1]
                        mtt = mttail_pool.tile([128, NKB], bf16, tag="mttail")

                        # software-pipelined at depth 2: AV_j issues after
                        # S_{j+2}, so the PE never waits on the scalar
                        # drain + DVE 2x-mode mask-multiply chain
                        def emit_s(j):
                            lhs_k = kT[b][r0 : r0 + 64, j * 128 : (j + 1) * 128]
                            st = ps_s.tile([128, QM], f32, tag="ps_s")
                            for (q0, qw) in st_slices():
                                nc.tensor.matmul(
                                    st[:, q0 : q0 + qw],
                                    lhs_k,
                                    qT[b][r0 : r0 + 64, q0 : q0 + qw],
                                    start=True, stop=True,
                                )
                            # tail column q=1024 (shares the kT weights)
                            nc.tensor.matmul(
                                stail[:, j : j + 1],
                                lhs_k,
                                qT[b][r0 : r0 + 64, QM : QM + 1],
                                start=True, stop=True,
                            )
                            # masked scores -> bf16.  Alternate per j: even
                            # j drains to bf16 on the scalar engine so the
                            # DVE multiply runs in 2x mode; odd j multiplies
                            # straight from PSUM at 1x.  This splits the
                            # ~1.1us/j chain across both engines so neither
                            # becomes the S-loop rate limiter.
                            mt = mt_pool.tile([128, QM], bf16, tag="mt")
                            if j % 2 == 0:
                                std = std_pool.tile([128, QM], bf16, tag="std")
                                nc.scalar.activation(std, st, AF.Copy)
                                nc.vector.tensor_mul(mt, std, mk[:, j, :])
                            else:
                                nc.vector.tensor_mul(mt, st, mk[:, j, :])
                            return mt

                        def emit_av(j, mt):
                            # AV accumulation (row 64 = rowsum via ones col)
                            for (q0, qw) in st_slices():
                                nc.tensor.matmul(
                                    av[:, q0 : q0 + qw],
                                    va[:, j, vc0 : vc0 + 65],
                                    mt[:, q0 : q0 + qw],
                                    start=(j == 0), stop=(j == NKB - 1),
                                )

                        mts = [emit_s(0), emit_s(1)]
                        for j in range(2, NKB):
                            mts.append(emit_s(j))
                            emit_av(j - 2, mts[j - 2])
                        emit_av(NKB - 2, mts[NKB - 2])
                        emit_av(NKB - 1, mts[NKB - 1])

                        # tail: masked scores + AV
                        nc.vector.tensor_mul(mtt, stail, mkt)
                        for j in range(NKB):
                            nc.tensor.matmul(
                                avt,
                                va[:, j, vc0 : vc0 + 65],
                                mtt[:, j : j + 1],
                                start=(j == 0), stop=(j == NKB - 1),
                            )

                        # drain AV psum to SBUF (frees the banks for the
                        # next head while the normalize chain runs)
                        av_sb = avsb_pool.tile([65, L], f32, tag="avsb")
                        nc.scalar.activation(av_sb[:, 0:QM], av, AF.Copy)
                        nc.scalar.activation(av_sb[:, QM : QM + 1], avt, AF.Copy)
                        av_sbs[(hh, b)] = av_sb

                # ---- batched normalization for the pair's 4 (hh, b) -----
                # gather rowsum rows into one tile, one batched reciprocal
                def normalize(combos, slot0):
                    n = len(combos)
                    rs = rs_pool.tile([4, L], f32, tag="rs")
                    for idx, (hh, b) in enumerate(combos):
                        dma2.dma_start(
                            out=rs[idx : idx + 1, :],
                            in_=av_sbs[(hh, b)][64:65, :],
                        )
                    nc.vector.tensor_scalar_add(
                        rs[0:n], rs[0:n], float(L) * EPS
                    )
                    rr = rs_pool.tile([4, L], f32, tag="rr")
                    nc.vector.reciprocal(rr[0:n], rs[0:n])
                    dma2.dma_start(out=rr_dram[slot0 : slot0 + n, :], in_=rr[0:n])
                    for idx, (hh, b) in enumerate(combos):
                        rr_slot = rr_dram[slot0 + idx]
                        rr_bcast_src = bass.AP(
                            tensor=rr_slot.tensor,
                            offset=rr_slot.offset,
                            ap=[[0, 64]] + list(rr_slot.ap),
                        )
                        rrb = rrb_pool.tile([64, L], f32, tag="rrb")
                        dma2.dma_start(out=rrb, in_=rr_bcast_src)
                        hg = (pair % 3) * 2 + hh
                        ci = b * 12 + g * 6 + hg
                        r0h = hh * 64
                        nc.vector.scalar_tensor_tensor(
                            ot_pairs[(b, pair)][r0h : r0h + 64, :],
                            av_sbs[(hh, b)][0:64, :],
                            cs_all[0:64, ci : ci + 1],
                            rrb,
                            op0=mybir.AluOpType.add,
                            op1=mybir.AluOpType.mult,
                        )

                if pair < 5:
                    normalize(
                        [(hh, b) for hh in range(2) for b in range(BPC)],
                        (pair % 2) * 4,
                    )
                else:
                    # last pair: per-batch so the O projection of b=0 isn't
                    # gated on b=1's normalize chain
                    normalize([(0, 0), (1, 0)], 4)
                    normalize([(0, 1), (1, 1)], 6)

            # ---- output projection: yT = wo^T @ O^T + bo ----------------
            ctx.close()
            ctx = octx.enter_context(ExitStack())
            y_pool = ctx.enter_context(tc.tile_pool(name="y", bufs=3))
            ps_y = ctx.enter_context(tc.tile_pool(name="ps_y", bufs=2, space="PSUM"))

            oq_tiles = [(0, 512), (512, 512), (1024, 1)]
            for b in range(BPC):
                for fc in range(6):
                    y_tile = y_pool.tile([128, L], bf16, tag="y", name="y_tile")
                    for (q0, qw) in oq_tiles:
                        psy = ps_y.tile([128, 512], f32, tag="ps_y")
                        for hc in range(6):
                            nc.tensor.matmul(
                                psy[:, 0:qw],
                                wo_sb[:, hc, fc * 128 : (fc + 1) * 128],
                                ot_pairs[(b, hc)][:, q0 : q0 + qw],
                                start=(hc == 0), stop=(hc == 5),
                            )
                        # drain with bo fused as the per-partition bias
                        nc.scalar.activation(
                            y_tile[:, q0 : q0 + qw], psy[:, 0:qw],
                            AF.Identity, bias=bo_sb[:, fc : fc + 1],
                        )
                    dma.dma_start(out=yT[b][:, fc, :], in_=y_tile)

    _split_matmul_waits(nc)
    return nc


def _split_matmul_waits(nc):
    """Walrus TPB instruction structs encode a limited number of sync waits
    (the fp32 LDWEIGHTS+MATMUL pair can take none beyond its update).  Hoist
    excess waits onto same-engine NoOps inserted just before each
    instruction."""
    import bass_rust
    from concourse import mybir

    n = 0
    for f in nc.m.functions:
        for blk in f.blocks:
            insts = blk.instructions
            out = []
            for inst in insts:
                si = inst.sync_info
                tname = type(inst).__name__
                if si is not None and len(si.on_wait) > 0 and tname != "InstISA":
                    cap = 0 if tname == "InstMatmult" else 1
                    waits = list(si.on_wait)
                    if len(waits) > cap:
                        hoist = waits[: len(waits) - cap]
                        keep = waits[len(waits) - cap :]
                        for w in hoist:
                            nop = mybir.InstNoOp(
                                name=f"I-mmw-{n}", ins=[], outs=[]
                            )
                            n += 1
                            nop.engine = inst.engine
                            nop.sync_info = bass_rust.SyncInfo(
                                on_wait=[w], on_update=[]
                            )
                            out.append(nop)
                        inst.sync_info = bass_rust.SyncInfo(
                            on_wait=keep, on_update=list(si.on_update)
                        )
                out.append(inst)
            insts[:] = out
    return n


def _dist_index():
    gi = np.arange(NB)
    gj = np.arange(NB)
    idx = (
        (gi[:, None, None, None] - gi[None, None, :, None] + NB) * 2 * NB
        + gj[None, :, None, None]
        - gj[None, None, None, :]
        + NB
    )
    return idx.reshape(-1).astype(np.int32)


def _host_prep(x, wq, bq, wk, bk, wv, bv, wo, bo, toeplitz_params):
    import ml_dtypes

    f4 = np.float32
    bf = ml_dtypes.bfloat16
    x = np.asarray(x, f4)
    L0 = NB * NB

    # x, transposed to [F, L], padded to LP, packed [128, 6, LP]
    xs = np.transpose(x, (0, 2, 1))  # [B, F, L]
    xaP = np.zeros((B, 128, 6, LP), bf)
    xaP[:, :, :, :L] = xs.reshape(B, 6, 128, L).transpose(0, 2, 1, 3).astype(bf)

    wq_flat = np.asarray(wq, f4).reshape(F, F)
    wk_flat = np.asarray(wk, f4).reshape(F, F)
    wqP = np.ascontiguousarray(
        wq_flat.reshape(6, 128, 6, 128).transpose(2, 1, 0, 3).astype(bf)
    )
    wkP = np.ascontiguousarray(
        wk_flat.reshape(6, 128, 6, 128).transpose(2, 1, 0, 3).astype(bf)
    )

    wvr = np.asarray(wv, f4)
    bvr = np.asarray(bv, f4)
    wv_aug = np.zeros((FA, H * 65), f4)
    for h in range(H):
        wv_aug[:F, h * 65 : h * 65 + 64] = wvr[:, h, :]
        wv_aug[F, h * 65 : h * 65 + 64] = bvr[h]
        wv_aug[F, h * 65 + 64] = 1.0
    wvP = np.zeros((2, 128, 7, 390), bf)
    wvP[:, :, :6, :] = (
        wv_aug[:F].reshape(6, 128, 2, 390).transpose(2, 1, 0, 3).astype(bf)
    )
    wvP[:, 0, 6, :] = wv_aug[F].reshape(2, 390).astype(bf)

    wo_flat = np.asarray(wo, f4).reshape(H * D, F)
    woP = np.ascontiguousarray(
        wo_flat.reshape(6, 128, F).transpose(1, 0, 2).astype(bf)
    )
    boP = np.ascontiguousarray(np.asarray(bo, f4).reshape(6, 128).T)

    bqs = (np.asarray(bq, f4).reshape(F) * 0.125).reshape(6, 128)
    bks = np.asarray(bk, f4).reshape(F).reshape(6, 128)
    bqkP = np.zeros((128, 12), f4)
    bqkP[:, 0::2] = bqs.T
    bqkP[:, 1::2] = bks.T

    # gathered |toeplitz| mask, padded (CLS row/col of ones), transposed,
    # k padded to 1152 with zeros, packed [H, 128, NKB, QM]
    tp = np.asarray(toeplitz_params, f4)
    tm = np.abs(tp[:, _dist_index()]).reshape(H, L0, L0)
    tm_full = np.ones((H, L, L), f4)
    tm_full[:, 1:, 1:] = tm
    maskT = np.zeros((H, LP, L), bf)
    maskT[:, :L, :] = np.transpose(tm_full, (0, 2, 1)).astype(bf)
    maskP = np.ascontiguousarray(
        maskT[:, :, :QM].reshape(H, NKB, 128, QM).transpose(0, 2, 1, 3)
    )
    maskT_tail = np.ascontiguousarray(
        maskT[:, :, QM].reshape(H, NKB, 128).transpose(0, 2, 1)
    )

    xsum = x.sum(axis=1)  # [B, F]
    cs = np.einsum("bf,fhd->bhd", xsum, wvr) + L * bvr[None]  # [B, H, 64]
    cs_full = np.concatenate(
        [cs, np.full((B, H, 1), float(L), np.float32)], axis=2
    ) * np.float32(EPS)  # [B, H, 65]

    shared = dict(
        bqkP=bqkP,
        wqP=wqP,
        wkP=wkP,
        wvP=wvP,
        woP=woP,
        boP=boP,
        maskP=maskP,
        maskT_tail=maskT_tail,
    )
    in_maps = []
    for c in range(NCORES):
        m = dict(shared)
        m["xaP"] = np.ascontiguousarray(xaP[c * BPC : (c + 1) * BPC])
        csP = np.zeros((65, 24), f4)
        for b in range(BPC):
            for g in range(2):
                for hg in range(6):
                    csP[:, b * 12 + g * 6 + hg] = cs_full[
                        c * BPC + b, 6 * g + hg, :
                    ]
        m["csP"] = csP
        in_maps.append(m)
    return in_maps


def _get_program():
    global _PROG
    if _PROG is None:
        _PROG = _build_program()
    return _PROG


def run(trace=False, **inputs):
    from concourse.bass_utils import run_bass_kernel_spmd

    nc = _get_program()
    in_maps = _host_prep(**inputs)
    res = run_bass_kernel_spmd(nc, in_maps, list(range(NCORES)), trace=trace)
    outs = []
    for c in range(NCORES):
        yt = np.asarray(res.results[c]["yT"], dtype=np.float32)  # [BPC,128,6,L]
        # y[b, l, fc*128 + p] = yt[b, p, fc, l]
        outs.append(yt.transpose(0, 3, 2, 1).reshape(BPC, L, F))
    y = np.concatenate(outs, axis=0).astype(np.float32)
    return y, res


def kernel(**inputs):
    y, _ = run(trace=False, **inputs)
    return y
